# revision 55
# baseline (speedup 1.0000x reference)
"""Trainium2 Bass kernel for nn_EnhancedFinGAT (4-layer GATv2 + GraphNorm + skip).

Strategy (8 NeuronCores, SPMD):
  - Nodes (and their incoming edges) are sharded by destination across the 8
    cores; per-core nodes are permuted into degree-bucket-major "slots" so the
    per-edge xr[dst] add becomes a broadcast access pattern (no second gather).
  - Per layer: local matmuls produce xl (normal layout, AllGathered into a
    DRAM table) and xr (transposed, SBUF).  Edge phase per 128-dst tile:
    one transposed dma_gather (for PE logits) + one normal dma_gather (for the
    PE mask-matmul weighted segment sum), leaky-relu via the |z| decomposition,
    softmax without max-subtraction (logits are bounded), exp on ScalarE,
    per-128-edge mask matmuls accumulate both the weighted sums and the
    softmax denominators in PSUM.
  - GraphNorm stats via one AllReduce of (sum, sum-of-squares); skip matmul in
    transposed layout.
All heavy per-edge data is bf16; accumulations are f32.
"""

import os
import sys
import numpy as np

sys.path.insert(0, "/opt/trn_rl_repo")

import concourse.bass as bass
import concourse.bacc as bacc
import concourse.mybir as mybir
import concourse.tile as tile
import concourse.bass_utils as bass_utils
from concourse.masks import make_identity

fp32 = mybir.dt.float32
f32r = mybir.dt.float32r
bf16 = mybir.dt.bfloat16
i16 = mybir.dt.int16

N, HID, L, H, CH = 10000, 256, 4, 4, 64
NCORES = 8
NPC = N // NCORES
EPS = 1e-5
P = 128


# ---------------------------------------------------------------- host prep

def _bucket_of(deg):
    if deg <= 128:
        return max(2, ((deg + 1) // 2) * 2)
    for k in (136, 144, 152, 160, 176, 192, 208, 224, 240, 256, 320, 384, 512):
        if deg <= k:
            return k
    raise ValueError(deg)


def _cumcount(x):
    n = len(x)
    if n == 0:
        return np.zeros(0, np.int64)
    change = np.empty(n, dtype=bool)
    change[0] = True
    change[1:] = x[1:] != x[:-1]
    run_starts = np.flatnonzero(change)
    return np.arange(n) - run_starts[np.cumsum(change) - 1]


def build_layout(edge_index, sub_cap=2048):
    src_g = np.concatenate([np.asarray(edge_index[0], np.int64), np.arange(N)])
    dst_g = np.concatenate([np.asarray(edge_index[1], np.int64), np.arange(N)])

    # degree-balanced node->core assignment: deal nodes (sorted by degree)
    # round-robin so the per-core bucket histograms almost coincide.
    deg_g = np.bincount(dst_g, minlength=N)
    order = np.argsort(-deg_g, kind="stable")
    core_nodes = [np.sort(order[c::NCORES]) for c in range(NCORES)]
    loc_of = np.zeros(N, np.int64)
    core_of_node = np.zeros(N, np.int64)
    for c in range(NCORES):
        loc_of[core_nodes[c]] = np.arange(NPC)
        core_of_node[core_nodes[c]] = c
    core_of = core_of_node[dst_g]

    per_core = []
    all_buckets = {}
    for c in range(NCORES):
        m = core_of == c
        s, d = src_g[m], loc_of[dst_g[m]]
        deg = np.bincount(d, minlength=NPC)
        buckets = np.array([_bucket_of(x) for x in deg])
        cnt = {}
        for k in buckets:
            cnt[int(k)] = cnt.get(int(k), 0) + 1
        for k, v in cnt.items():
            all_buckets[k] = max(all_buckets.get(k, 0), v)
        per_core.append((s, d, buckets))

    ks = sorted(all_buckets)
    M = {k: all_buckets[k] for k in ks}
    NL = ((sum(M.values()) + P - 1) // P) * P
    NT = NCORES * NL
    PAD_ROW = 0

    slot_bucket = np.zeros(NL, np.int64)
    off = 0
    bucket_slot_base = {}
    for k in ks:
        bucket_slot_base[k] = off
        slot_bucket[off:off + M[k]] = k
        off += M[k]

    n_tiles = NL // P
    SUB_CAP = sub_cap
    slot_edge_off = np.zeros(NL, np.int64)
    tile_edge_base = np.zeros(n_tiles + 1, np.int64)
    tile_subs = []  # per tile: list of (e_start, e_end, [(k, d0, m), ...])
    e = 0
    for t in range(n_tiles):
        tile_edge_base[t] = e
        subs = []
        sub_start = e
        sub_runs = []
        run = None  # (k, d0, m)
        for d in range(t * P, (t + 1) * P):
            k = int(slot_bucket[d])
            if k == 0:
                continue
            pad_now = ((e + P - 1) // P) * P
            if pad_now + k - sub_start > SUB_CAP:
                # close current sub before this node
                if run is not None:
                    sub_runs.append(run)
                    run = None
                e = pad_now
                subs.append((int(sub_start), int(e), sub_runs))
                sub_runs = []
                sub_start = e
            slot_edge_off[d] = e
            if run is not None and run[0] == k:
                run = (k, run[1], run[2] + 1)
            else:
                if run is not None:
                    sub_runs.append(run)
                run = (k, d, 1)
            e += k
        if run is not None:
            sub_runs.append(run)
        if e > sub_start or sub_runs:
            e = ((e + P - 1) // P) * P
            subs.append((int(sub_start), int(e), sub_runs))
        tile_subs.append(subs)
    tile_edge_base[n_tiles] = e
    E_pad = int(e)

    cores = []
    for c in range(NCORES):
        s, d, buckets = per_core[c]
        slot_of_node = np.full(NPC, -1, np.int64)
        next_free = dict(bucket_slot_base)
        for n_loc in np.argsort(buckets, kind="stable"):
            k = int(buckets[n_loc])
            slot_of_node[n_loc] = next_free[k]
            next_free[k] += 1
        order = np.argsort(slot_of_node[d], kind="stable")
        cores.append(dict(slot_of_node=slot_of_node,
                          s_sorted=s[order],
                          d_sorted_slot=slot_of_node[d][order]))

    g2p = np.zeros(N, np.int64)
    for c in range(NCORES):
        g2p[core_nodes[c]] = c * NL + cores[c]["slot_of_node"]

    # run membership arrays: edges of a run (k, d0, m) are interleaved
    # dst-major (edge j of dst i at run_base + j*m + i) so the xr[dst]
    # broadcast add has a packed last dim (DVE 2x mode).
    run_d0 = np.arange(NL)
    run_m = np.ones(NL, np.int64)
    for subs in tile_subs:
        for (_e0, _e1, runs) in subs:
            for (k, d0, m) in runs:
                run_d0[d0:d0 + m] = d0
                run_m[d0:d0 + m] = m

    for c in range(NCORES):
        src_slot = np.full(E_pad, PAD_ROW, np.int64)
        dst_slot = np.full(E_pad, -1, np.int64)
        d_sl = cores[c]["d_sorted_slot"]
        pos = (slot_edge_off[run_d0[d_sl]] + _cumcount(d_sl) * run_m[d_sl]
               + (d_sl - run_d0[d_sl]))
        src_slot[pos] = g2p[cores[c]["s_sorted"]]
        dst_slot[pos] = d_sl
        cores[c]["src_slot"] = src_slot
        cores[c]["dst_slot_of_edge"] = dst_slot

    return dict(NL=int(NL), NT=int(NT), PAD_ROW=int(PAD_ROW), E_pad=E_pad,
                n_tiles=n_tiles, tile_edge_base=tile_edge_base,
                tile_subs=tile_subs, slot_edge_off=slot_edge_off,
                core_nodes=core_nodes, g2p=g2p, sub_cap=int(SUB_CAP)), cores


def wrap_idx16(idx):
    n = len(idx)
    cols = (n + 15) // 16
    pad = np.zeros(cols * 16, np.int64)
    pad[:n] = idx
    w = pad.reshape(cols, 16).T.astype(np.int16)
    return np.tile(w, (8, 1))


def build_masks(layout, core):
    E_pad = layout["E_pad"]
    n_chunks = E_pad // P
    dst = core["dst_slot_of_edge"]
    masks = np.zeros((n_chunks, P, P), np.float32)
    for chn in range(n_chunks):
        d = dst[chn * P:(chn + 1) * P]
        rows = np.flatnonzero(d >= 0)
        masks[chn, rows, (d[d >= 0] % P)] = 1.0
    return masks


def att4_lhst(att_l, scale):
    # features are channel-interleaved: feature f holds (h, c) = (f%4, f//4)
    out = np.zeros((2, P, P), np.float32)
    for b in range(2):
        for p in range(P):
            f = 128 * b + p
            h = f % H
            out[b, p, h::4] = scale * att_l[h, f // H]
    return out


def _to_bf16(x):
    import jax.numpy as jnp
    return np.asarray(jnp.asarray(x, jnp.bfloat16)).view(np.uint16)


# numpy bf16 arrays are passed as uint16 views?  Simpler: use ml_dtypes.
def to_bf16(x):
    import ml_dtypes
    return np.asarray(x, np.float32).astype(ml_dtypes.bfloat16)


def prep_inputs(inputs, sub_cap=2048):
    """Returns (layout, in_maps) — one dict per core."""
    layout, cores = build_layout(np.asarray(inputs["edge_index"]), sub_cap=sub_cap)
    NL, E_pad = layout["NL"], layout["E_pad"]

    x = np.asarray(inputs["x"], np.float32)
    lw = np.asarray(inputs["lin_l_w"], np.float32)
    lb = np.asarray(inputs["lin_l_b"], np.float32)
    rw = np.asarray(inputs["lin_r_w"], np.float32)
    rb = np.asarray(inputs["lin_r_b"], np.float32)
    att = np.asarray(inputs["att"], np.float32)
    cb = np.asarray(inputs["conv_bias"], np.float32)
    gnw = np.asarray(inputs["gn_weight"], np.float32)
    gnb = np.asarray(inputs["gn_bias"], np.float32)
    gnm = np.asarray(inputs["gn_mean_scale"], np.float32)
    skw = np.asarray(inputs["skip_w"], np.float32)

    # channel-interleaved h-space: new feature c*H+h <- old h*CH+c
    PERM = np.zeros(HID, np.int64)
    for h in range(H):
        PERM[np.arange(CH) * H + h] = h * CH + np.arange(CH)
    lw = lw[:, :, PERM]
    lb = lb[:, PERM]
    rw = rw[:, :, PERM]
    rb = rb[:, PERM]
    cb = cb[:, PERM]
    gnw = gnw[:, PERM]
    gnb = gnb[:, PERM]
    gnm = gnm[:, PERM]
    skw = skw[PERM, :]
    skb = np.asarray(inputs["skip_b"], np.float32)
    fcw = np.asarray(inputs["fc_w"], np.float32)
    fcb = np.asarray(inputs["fc_b"], np.float32)

    # layer-stacked common weights (same on all cores)
    wl_in = lw.reshape(L, 2, 128, 256)                    # [l, bi, 128, 256]
    wr_in = rw.reshape(L, 2, 128, 256)
    wsk_in = skw.reshape(2, 128, 2, 128).transpose(0, 2, 1, 3)  # [bi, bo, 128, 128]
    bl_in = lb.reshape(L, 1, 256)
    br_t = rb.reshape(L, 2, 128, 1)
    bsk_eff = (skb - skw.sum(axis=0)).reshape(2, 128, 1)
    a4z = np.stack([att4_lhst(att[l], 0.6) for l in range(L)])  # [L, 2, 128, 128]
    a4a = np.stack([att4_lhst(att[l], 0.4) for l in range(L)])
    cbA = (cb * (1.0 - gnm)).reshape(L, 2, 128, 1)
    cbB = np.broadcast_to((-gnm / float(N)).reshape(L, 2, 128, 1), (L, 2, 128, 1))
    gnw_t = gnw.reshape(L, 2, 128, 1)
    gnb_t = gnb.reshape(L, 2, 128, 1)
    fcw_t = fcw.reshape(2, 128, 1)
    fcb_in = fcb.reshape(1, 1)

    identj = np.zeros((128, 16), np.float32)
    for j in range(3):
        identj[32 * j:32 * j + 16] = np.eye(16)

    common = dict(
        wl=to_bf16(wl_in), wr=to_bf16(wr_in), wsk=to_bf16(wsk_in.copy()),
        bl=to_bf16(bl_in), br_t=br_t,
        bsk=bsk_eff, a4z=to_bf16(a4z), a4a=to_bf16(a4a),
        cbA=cbA, cbB=np.asarray(cbB, np.float32).copy(), gnw_t=gnw_t, gnb_t=gnb_t,
        fcw_t=to_bf16(fcw_t), fcb=fcb_in, identj=to_bf16(identj),
    )

    in_maps = []
    for c in range(NCORES):
        core = cores[c]
        x0 = np.zeros((NL, HID), np.float32)
        x0[core["slot_of_node"]] = x[layout["core_nodes"][c]]
        x0t = x0.T.reshape(2, 128, NL).copy()
        m = dict(common)
        m["x0t"] = x0t
        m["iotaw"] = wrap_idx16(np.arange(layout["sub_cap"]))
        m["srcw"] = wrap_idx16(core["src_slot"])
        d2 = core["dst_slot_of_edge"].reshape(-1, 128).T
        m["dsts"] = np.where(d2 >= 0, d2 % 128, -1).astype(np.float32)
        in_maps.append(pack_core(layout, m))

    layout["cores"] = cores
    return layout, in_maps


def pack_spec(layout):
    """Fixed packing of all external inputs into 3 dtype buffers.
    Returns name -> (buf_key, offset, shape); buf keys: pb (bf16), pf (f32),
    pi (i16)."""
    NL, E_pad = layout["NL"], layout["E_pad"]
    EW = (E_pad + 15) // 16
    NCH = E_pad // P
    sc = layout["sub_cap"]
    specs = {}
    offs = {"pb": 0, "pf": 0, "pi": 0}

    def add(key, name, shape):
        specs[name] = (key, offs[key], tuple(shape))
        offs[key] += int(np.prod(shape))

    add("pb", "wl", (L, 2, 128, 256))
    add("pb", "wr", (L, 2, 128, 256))
    add("pb", "wsk", (2, 2, 128, 128))
    add("pb", "bl", (L, 1, 256))
    add("pb", "a4z", (L, 2, 128, 128))
    add("pb", "a4a", (L, 2, 128, 128))
    add("pb", "fcw_t", (2, 128, 1))
    add("pb", "identj", (128, 16))
    add("pf", "x0t", (2, 128, NL))
    add("pf", "br_t", (L, 2, 128, 1))
    add("pf", "bsk", (2, 128, 1))
    add("pf", "cbA", (L, 2, 128, 1))
    add("pf", "cbB", (L, 2, 128, 1))
    add("pf", "gnw_t", (L, 2, 128, 1))
    add("pf", "gnb_t", (L, 2, 128, 1))
    add("pf", "fcb", (1, 1))
    add("pf", "dsts", (128, NCH))
    add("pi", "srcw", (128, EW))
    add("pi", "iotaw", (128, sc // 16))
    return specs, dict(offs)


def pack_core(layout, tensors):
    import ml_dtypes
    specs, sizes = pack_spec(layout)
    out = {
        "pb": np.zeros((1, sizes["pb"]), ml_dtypes.bfloat16),
        "pf": np.zeros((1, sizes["pf"]), np.float32),
        "pi": np.zeros((1, sizes["pi"]), np.int16),
    }
    for name, (key, off, shape) in specs.items():
        arr = tensors[name]
        assert tuple(arr.shape) == tuple(shape), (name, arr.shape, shape)
        out[key][0, off:off + arr.size] = np.asarray(arr).reshape(-1)
    return out


# ---------------------------------------------------------------- device build

def build_program(layout, n_layers=L, do_edges=True, do_coll=True, edge_stage=5,
                  do_gt=True, do_gn=True, do_mk=True, spk=False, gt_from_sbuf=False,
                  gt_from_table=False, alt_queues=False, split_q=False, gn_from_gt=False,
                  dma_scratch=16384):
    NL, NT, E_pad = layout["NL"], layout["NT"], layout["E_pad"]
    n_tiles = layout["n_tiles"]
    teb = layout["tile_edge_base"]
    EW = (E_pad + 15) // 16
    NCH = E_pad // P
    SUBMAX = 128 * max(
        (s[1] - s[0]) // 128 for subs in layout["tile_subs"] for s in subs)

    nc = bacc.Bacc("TRN2", target_bir_lowering=False, debug=False,
                   num_devices=NCORES, num_swdge_queues=4 if (alt_queues or split_q) else 2,
                   dynamic_dma_scratch_size=dma_scratch)

    # ---- I/O: all external inputs packed into 3 dtype buffers
    specs, sizes = pack_spec(layout)
    pb = nc.dram_tensor("pb", [1, sizes["pb"]], bf16, kind="ExternalInput")
    pf = nc.dram_tensor("pf", [1, sizes["pf"]], fp32, kind="ExternalInput")
    pi = nc.dram_tensor("pi", [1, sizes["pi"]], i16, kind="ExternalInput")
    bufs = {"pb": pb, "pf": pf, "pi": pi}

    def pslice(name, idx=None):
        """Flat [1, numel] AP of tensor `name`, optionally of slice [idx]
        along dim 0."""
        key, off, shape = specs[name]
        numel = int(np.prod(shape))
        if idx is None:
            return bufs[key][0:1, off:off + numel]
        sub = numel // shape[0]
        return bufs[key][0:1, off + idx * sub:off + (idx + 1) * sub]

    def v_pbc(name, idx, b, p_, c):      # [l] . "b p c -> p b c"
        return pslice(name, idx).rearrange("o (b p c) -> (o p) b c", b=b, p=p_, c=c)

    def v_pb2(name, idx=None):           # [l] . "b p o -> p (b o)" (o==1)
        return pslice(name, idx).rearrange("z (b p) -> (z p) b", b=2, p=128)

    x0t_ap = lambda b: pslice("x0t", b).rearrange("o (p n) -> (o p) n", p=128)
    srcw_ap = pslice("srcw").rearrange("o (p e) -> (o p) e", p=128)
    iotaw_ap = pslice("iotaw").rearrange("o (p e) -> (o p) e", p=128)
    dsts_ap = pslice("dsts").rearrange("o (p n) -> (o p) n", p=128, n=NCH)
    identj_ap = pslice("identj").rearrange("o (p c) -> (o p) c", p=128)
    fcb_ap = pslice("fcb")

    y_out = nc.dram_tensor("y", [1, NL], fp32, kind="ExternalOutput")

    # ---- internal DRAM
    xl_bounce = nc.dram_tensor("xl_bounce", [NL, 256], bf16, kind="Internal")
    masks_dram = nc.dram_tensor("masks_dram", [128, NCH, 128], bf16, kind="Internal")
    st_in = nc.dram_tensor("st_in", [256, 2], fp32, kind="Internal")
    st_out = nc.dram_tensor("st_out", [256, 2], fp32, kind="Internal",
                            addr_space="Shared")
    xlt_sh = nc.dram_tensor("xlt_sh", [NT, 256], bf16, kind="Internal",
                            addr_space="Shared")

    groups = [list(range(NCORES))]

    with tile.TileContext(nc) as tc:
        with tc.tile_pool(name="persist", bufs=1) as pp, \
             tc.tile_pool(name="work", bufs=2) as wp, \
             tc.tile_pool(name="single", bufs=1) as sp, \
             tc.tile_pool(name="psum", bufs=2, space="PSUM") as psp:

            # ---------------- constants / persistent state
            ident128 = pp.tile([128, 128], fp32, tag="id128", name="id128")
            make_identity(nc, ident128[:])
            ident128b = pp.tile([128, 128], bf16, tag="id128b", name="id128b")
            nc.vector.tensor_copy(ident128b[:], ident128[:])
            identj_sb = pp.tile([128, 16], bf16, tag="idj", name="idj")
            nc.sync.dma_start(out=identj_sb[:], in_=identj_ap)

            ones_row = pp.tile([1, 128], bf16, tag="ones", name="ones")
            eps_t = pp.tile([128, 1], fp32, tag="epsT", name="epsT")
            nc.gpsimd.memset(eps_t[:], EPS)
            nc.gpsimd.memset(ones_row[:], 1.0)

            xT = [pp.tile([128, NL], fp32, tag=f"xT{b}", name=f"xT{b}") for b in range(2)]
            xrT = [pp.tile([128, NL], bf16, tag=f"xrT{b}", name=f"xrT{b}") for b in range(2)]
            xrTf = [pp.tile([128, NL], fp32, tag=f"xrTf{b}", name=f"xrTf{b}") for b in range(2)]
            xTb = [pp.tile([128, NL], bf16, tag=f"xTb{b}", name=f"xTb{b}") for b in range(2)]
            outT = [pp.tile([128, NL], fp32, tag=f"outT{b}", name=f"outT{b}") for b in range(2)]
            hT = [pp.tile([128, NL], bf16, tag=f"hT{b}", name=f"hT{b}") for b in range(2)]
            xl_sb = pp.tile([128, (NL // 128) * 256], bf16, tag="xlsb", name="xlsb")
            xlt_sb = (pp.tile([128, NT // 128, 256], bf16, tag="xltsb", name="xltsb")
                      if gt_from_table else None)
            srcw_sb = pp.tile([128, EW], i16, tag="srcsb", name="srcsb")
            nc.sync.dma_start(out=srcw_sb[:], in_=srcw_ap)
            iotaw_sb = pp.tile([128, layout["sub_cap"] // 16], i16, tag="iotasb", name="iotasb")
            nc.sync.dma_start(out=iotaw_sb[:], in_=iotaw_ap)
            dsts_sb = pp.tile([128, NCH], fp32, tag="dstssb", name="dstssb")
            nc.sync.dma_start(out=dsts_sb[:], in_=dsts_ap)
            iota_d = pp.tile([128, 128], fp32, tag="iotad", name="iotad")
            nc.gpsimd.iota(iota_d[:], pattern=[[1, 128]], base=0,
                           channel_multiplier=0,
                           allow_small_or_imprecise_dtypes=True)

            for b in range(2):
                nc.sync.dma_start(out=xT[b][:], in_=x0t_ap(b))
                nc.vector.tensor_copy(xTb[b][:], xT[b][:])

            # per-layer weight staging
            wl_s = pp.tile([128, 2, 256], bf16, tag="wls", name="wls")
            wr_s = pp.tile([128, 2, 256], bf16, tag="wrs", name="wrs")
            wsk_s = pp.tile([128, 2, 2, 128], bf16, tag="wsks", name="wsks")
            bl_s = pp.tile([1, 256], bf16, tag="bls", name="bls")
            br_s = pp.tile([128, 2], fp32, tag="brs", name="brs")
            bsk_s = pp.tile([128, 2], fp32, tag="bsks", name="bsks")
            a4z_s = pp.tile([128, 2, 128], bf16, tag="a4zs", name="a4zs")
            a4a_s = pp.tile([128, 2, 128], bf16, tag="a4as", name="a4as")
            cbA_s = pp.tile([128, 2], fp32, tag="cbAs", name="cbAs")
            cbB_s = pp.tile([128, 2], fp32, tag="cbBs", name="cbBs")
            gnw_s = pp.tile([128, 2], fp32, tag="gnws", name="gnws")
            gnb_s = pp.tile([128, 2], fp32, tag="gnbs", name="gnbs")

            for b in range(2):
                nc.sync.dma_start(out=wsk_s[:, b], in_=pslice("wsk", b).rearrange("z (o p c) -> (z p) o c", o=2, p=128, c=128))
            nc.sync.dma_start(out=bsk_s[:], in_=v_pb2("bsk"))

            def layer(l):
                # ---- stage layer weights
                nc.sync.dma_start(out=wl_s[:], in_=v_pbc("wl", l, 2, 128, 256))
                nc.sync.dma_start(out=wr_s[:], in_=v_pbc("wr", l, 2, 128, 256))
                nc.sync.dma_start(out=bl_s[:], in_=pslice("bl", l))
                nc.sync.dma_start(out=br_s[:], in_=v_pb2("br_t", l))
                nc.sync.dma_start(out=a4z_s[:], in_=v_pbc("a4z", l, 2, 128, 128))
                nc.sync.dma_start(out=a4a_s[:], in_=v_pbc("a4a", l, 2, 128, 128))
                nc.sync.dma_start(out=cbA_s[:], in_=v_pb2("cbA", l))
                nc.sync.dma_start(out=cbB_s[:], in_=v_pb2("cbB", l))
                nc.sync.dma_start(out=gnw_s[:], in_=v_pb2("gnw_t", l))
                nc.sync.dma_start(out=gnb_s[:], in_=v_pb2("gnb_t", l))

                # ---- xl (normal layout) and xr (transposed) from x_T
                for t in range(NL // 128):
                    ps = psp.tile([128, 512], fp32, tag="lg", name="lg", bufs=3)
                    for bi in range(2):
                        nc.tensor.matmul(out=ps[:, :256],
                                         lhsT=xTb[bi][:, t * 128:(t + 1) * 128],
                                         rhs=wl_s[:, bi],
                                         start=(bi == 0), stop=False)
                    nc.tensor.matmul(out=ps[:, :256], lhsT=ones_row[:],
                                     rhs=bl_s[:], start=False, stop=True)
                    nc.vector.tensor_copy(
                        xl_sb[:, t * 256:(t + 1) * 256], ps[:, :256])
                nc.sync.dma_start(
                    out=xl_bounce[:].rearrange("(t p) c -> p t c", p=128),
                    in_=xl_sb[:].rearrange("p (t c) -> p t c", c=256))

                # xr transposed: out block bo over node chunks of 512
                for bo in range(2):
                    for ch0 in range(0, NL, 512):
                        cw = min(512, NL - ch0)
                        ps = psp.tile([128, 512], fp32, tag="wsum", name="wsum")
                        for bi in range(2):
                            nc.tensor.matmul(
                                out=ps[:, :cw],
                                lhsT=wr_s[:, bi, bo * 128:(bo + 1) * 128],
                                rhs=xTb[bi][:, ch0:ch0 + cw],
                                start=(bi == 0), stop=(bi == 1))
                        nc.scalar.activation(
                            out=xrT[bo][:, ch0:ch0 + cw], in_=ps[:, :cw],
                            func=mybir.ActivationFunctionType.Identity,
                            bias=br_s[:, bo:bo + 1], scale=1.0)
                        nc.scalar.activation(
                            out=xrTf[bo][:, ch0:ch0 + cw], in_=ps[:, :cw],
                            func=mybir.ActivationFunctionType.Identity,
                            bias=br_s[:, bo:bo + 1], scale=1.0)

                # ---- AllGather xl into the table
                if do_coll:
                    nc.gpsimd.collective_compute(
                        "AllGather", mybir.AluOpType.bypass,
                        replica_groups=groups,
                        ins=[xl_bounce[:]],
                        outs=[xlt_sh[:]],
                    )
                if gt_from_table:
                    nc.sync.dma_start(
                        out=xlt_sb[:],
                        in_=xlt_sh[:].rearrange("(t p) c -> p t c", p=128))

                # ---- edge phase, per dst-tile, sub-chunked
                sub_seq = [0]
                cpi = [0]
                if not do_edges or edge_stage < 5:
                    for b in range(2):
                        nc.gpsimd.memset(outT[b][:], 0.0)
                # software-pipelined emission: defer each sub's B-phase
                # (wg + mask matmuls + tile-post) by PIPE_LAG subs so in-order
                # engines don't stall on the long gt->wn chain.
                PIPE_LAG = 4
                pend = []
                pend_wn = []

                def flush_b():
                    d = pend.pop(0)
                    nch_ = d["nch"]
                    pw_ = d["pw"]
                    if edge_stage >= 4:
                        wg = wp.tile([128, nch_, 260], bf16, tag="wg", name="wg",
                                     padded_shape=[128, SUBMAX // 128, 260],
                                     bufs=3)
                        nc.vector.tensor_tensor(
                            out=wg[:, :, 0:256].rearrange("p n (c f) -> p n c f", f=4),
                            in0=d["gn"][:].rearrange("p n (c f) -> p n c f", f=4),
                            in1=d["wn"][:, :, 0:4].to_broadcast([128, nch_, 4, 64])
                                .rearrange("p n f c -> p n c f"),
                            op=mybir.AluOpType.mult)
                        nc.vector.tensor_copy(wg[:, :, 256:260], d["wn"][:, :, 0:4])
                        for n in range(nch_ if edge_stage >= 5 else 0):
                            nc.tensor.matmul(out=pw_[:, :260], lhsT=d["mk"][:, n],
                                             rhs=wg[:, n],
                                             start=(d["ci0"] + n == 0),
                                             stop=(d["ci0"] + n == d["nct"] - 1))
                    if d["last"] and edge_stage >= 5:
                        t_ = d["t"]
                        srec = wp.tile([128, 4], fp32, tag="srec", name="srec")
                        nc.vector.tensor_scalar(
                            out=srec[:], in0=pw_[:, 256:260], scalar1=1e-20,
                            scalar2=None, op0=mybir.AluOpType.add)
                        nc.vector.reciprocal(srec[:], srec[:])
                        outn = wp.tile([128, 256], fp32, tag="outn", name="outn")
                        nc.vector.tensor_tensor(
                            out=outn[:].rearrange("p (c f) -> p c f", f=4),
                            in0=pw_[:, 0:256].rearrange("p (c f) -> p c f", f=4),
                            in1=srec[:].to_broadcast([128, 4, 64])
                                .rearrange("p f c -> p c f"),
                            op=mybir.AluOpType.mult)
                        for b in range(2):
                            tp = psp.tile([128, 128], fp32, tag="tpo", name="tpo", bufs=1)
                            nc.tensor.transpose(
                                out=tp[:], in_=outn[:, b * 128:(b + 1) * 128],
                                identity=ident128[:])
                            nc.vector.tensor_copy(
                                outT[b][:, t_ * 128:(t_ + 1) * 128], tp[:])

                for t in range(n_tiles if do_edges else 0):
                    subs = layout["tile_subs"][t]
                    n_sub = len(subs)
                    chunk_i = 0
                    n_chunks_tile = sum((s[1] - s[0]) // 128 for s in subs)
                    for si, (e0, e1, runs) in enumerate(subs):
                        et = e1 - e0
                        nch = et // 128
                        if si == 0:
                            pw = psp.tile([128, 512], fp32, tag="wsum", name="wsum")
                        gbufs = {512: 6, 1024: 4, 2048: 3}.get(layout["sub_cap"], 2)
                        gt = wp.tile([128, 2 * et], bf16, tag="gt", name="gt",
                                     padded_shape=[128, 2 * SUBMAX], bufs=gbufs)
                        gn = wp.tile([128, nch, 256], bf16, tag="gn", name="gn",
                                     padded_shape=[128, SUBMAX // 128, 256],
                                     bufs=gbufs + 1)
                        mk = wp.tile([128, nch, 128], bf16, tag="mk", name="mk",
                                     padded_shape=[128, SUBMAX // 128, 128],
                                     bufs=gbufs + 1)
                        za = wp.tile([128, 2 * et], bf16, tag="za", name="za",
                                     padded_shape=[128, 2 * SUBMAX], bufs=2)
                        w16 = wp.tile([16, et], bf16, tag="w16", name="w16",
                                      padded_shape=[16, SUBMAX], bufs=gbufs + 1)
                        wn = wp.tile([128, nch, 16], bf16, tag="wn", name="wn",
                                     padded_shape=[128, SUBMAX // 128, 16],
                                     bufs=gbufs + 1)

                        qbase = (sub_seq[0] % 2) * 2 if alt_queues else 0
                        sub_seq[0] += 1
                        if do_gt and not gt_from_sbuf and not gt_from_table:
                            if split_q:
                                for b in range(2):
                                    nc.gpsimd.dma_gather(
                                        out_ap=gt[:, b * et:(b + 1) * et].rearrange("p (u e) -> p u e", u=1),
                                        in_ap=xlt_sh[:, b * 128:(b + 1) * 128],
                                        idxs_ap=srcw_sb[:, e0 // 16:e1 // 16],
                                        num_idxs=et, num_idxs_reg=et,
                                        elem_size=128, elem_step=256,
                                        transpose=True, single_packet=spk,
                                        queue_num=2 * b)
                            else:
                                nc.gpsimd.dma_gather(
                                    out_ap=gt[:].rearrange("p (b e) -> p b e", b=2), in_ap=xlt_sh[:],
                                    idxs_ap=srcw_sb[:, e0 // 16:e1 // 16],
                                    num_idxs=et, num_idxs_reg=et, elem_size=256,
                                    transpose=True, single_packet=spk,
                                    queue_num=qbase)
                        if do_gt and gt_from_table:
                            nc.gpsimd.dma_gather(
                                out_ap=gt[:].rearrange("p (b e) -> p b e", b=2),
                                in_ap=xlt_sb[:],
                                idxs_ap=srcw_sb[:, e0 // 16:e1 // 16],
                                num_idxs=et, num_idxs_reg=et, elem_size=256,
                                transpose=True, single_packet=spk,
                                sbuf_tokens_per_rank=128,
                                sbuf_free_dim_per_rank=512,
                                sbuf_free_dim_pad_per_rank=0,
                                sbuf_byte_offset=0)
                        if do_gt and gt_from_sbuf:
                            nc.gpsimd.dma_gather(
                                out_ap=gt[:].rearrange("p (b e) -> p b e", b=2),
                                in_ap=gn[:],
                                idxs_ap=iotaw_sb[:, :et // 16],
                                num_idxs=et, num_idxs_reg=et, elem_size=256,
                                transpose=True, single_packet=spk,
                                sbuf_tokens_per_rank=128,
                                sbuf_free_dim_per_rank=512,
                                sbuf_free_dim_pad_per_rank=0,
                                sbuf_byte_offset=0)
                        if do_gn and gn_from_gt:
                            for g4 in range(0, nch, 4):
                                gw4 = min(4, nch - g4)
                                for b in range(2):
                                    tpg = psp.tile([128, 512], bf16, tag="tpg",
                                                   name="tpg")
                                    for k in range(gw4):
                                        n = g4 + k
                                        nc.tensor.transpose(
                                            out=tpg[:, k * 128:(k + 1) * 128],
                                            in_=gt[:, b * et + n * 128:
                                                   b * et + (n + 1) * 128],
                                            identity=ident128b[:])
                                    dst_ap = gn[:, g4:g4 + gw4,
                                                b * 128:(b + 1) * 128]
                                    src_ap = tpg[:, :gw4 * 128].rearrange(
                                        "p (k c) -> p k c", k=gw4)
                                    if cpi[0] % 2 == 0:
                                        nc.vector.tensor_copy(dst_ap, src_ap)
                                    else:
                                        nc.scalar.activation(
                                            out=dst_ap, in_=src_ap,
                                            func=mybir.ActivationFunctionType.Copy)
                                    cpi[0] += 1
                        if do_gn and not gn_from_gt:
                            if split_q and et >= 256:
                                eh0 = (nch // 2) * 128
                                for (s0, s1, q) in ((0, eh0, 1), (eh0, et, 3)):
                                    nc.gpsimd.dma_gather(
                                        out_ap=gn[:, s0 // 128:s1 // 128],
                                        in_ap=xlt_sh[:],
                                        idxs_ap=srcw_sb[:, (e0 + s0) // 16:(e0 + s1) // 16],
                                        num_idxs=s1 - s0, num_idxs_reg=s1 - s0,
                                        elem_size=256,
                                        transpose=False, single_packet=spk,
                                        queue_num=q)
                            else:
                                nc.gpsimd.dma_gather(
                                    out_ap=gn[:], in_ap=xlt_sh[:],
                                    idxs_ap=srcw_sb[:, e0 // 16:e1 // 16],
                                    num_idxs=et, num_idxs_reg=et, elem_size=256,
                                    transpose=False, single_packet=spk,
                                    queue_num=qbase + 1)
                        if do_mk:
                            if l == 0:
                                nc.vector.tensor_tensor(
                                    out=mk[:],
                                    in0=dsts_sb[:, e0 // 128:e1 // 128]
                                        .to_broadcast([128, nch, 128]),
                                    in1=iota_d[:].rearrange("p (u c) -> p u c", u=1)
                                        .to_broadcast([128, nch, 128]),
                                    op=mybir.AluOpType.is_equal)
                                nc.sync.dma_start(
                                    out=masks_dram[:, e0 // 128:e1 // 128, :],
                                    in_=mk[:])
                            else:
                                nc.sync.dma_start(
                                    out=mk[:],
                                    in_=masks_dram[:, e0 // 128:e1 // 128, :])

                        # z = g + xr[dst] via bucket-broadcast; pad tail stays g
                        for b in range(2 if edge_stage >= 2 else 0):
                            for (k, d0, m) in runs:
                                off = int(layout["slot_edge_off"][d0] - e0)
                                base = b * et + off
                                if m == 1:
                                    nc.vector.tensor_scalar(
                                        out=gt[:, base:base + k],
                                        in0=gt[:, base:base + k],
                                        scalar1=xrTf[b][:, d0:d0 + 1],
                                        scalar2=None, op0=mybir.AluOpType.add)
                                else:
                                    seg = gt[:, base:base + m * k] \
                                        .rearrange("p (k m) -> p k m", m=m)
                                    nc.vector.tensor_tensor(
                                        out=seg, in0=seg,
                                        in1=xrT[b][:, d0:d0 + m]
                                            .to_broadcast([128, m, k])
                                            .rearrange("p m k -> p k m"),
                                        op=mybir.AluOpType.add)
                            nc.scalar.activation(
                                out=za[:, b * et:(b + 1) * et],
                                in_=gt[:, b * et:(b + 1) * et],
                                func=mybir.ActivationFunctionType.Abs)

                        # logits + exp, 512-edge chunks
                        for ch0 in range(0, et if edge_stage >= 3 else 0, 512):
                            cw = min(512, et - ch0)
                            lg = psp.tile([128, 512], fp32, tag="lg", name="lg", bufs=3)
                            nc.tensor.matmul(out=lg[:16, :cw], lhsT=a4z_s[:, 0, 0:16],
                                             rhs=gt[:, ch0:ch0 + cw],
                                             start=True, stop=False)
                            nc.tensor.matmul(out=lg[:16, :cw], lhsT=a4z_s[:, 1, 0:16],
                                             rhs=gt[:, et + ch0:et + ch0 + cw],
                                             start=False, stop=False)
                            nc.tensor.matmul(out=lg[:16, :cw], lhsT=a4a_s[:, 0, 0:16],
                                             rhs=za[:, ch0:ch0 + cw],
                                             start=False, stop=False)
                            nc.tensor.matmul(out=lg[:16, :cw], lhsT=a4a_s[:, 1, 0:16],
                                             rhs=za[:, et + ch0:et + ch0 + cw],
                                             start=False, stop=True)
                            nc.scalar.activation(
                                out=w16[:16, ch0:ch0 + cw], in_=lg[:16, :cw],
                                func=mybir.ActivationFunctionType.Exp)

                        if edge_stage >= 3:
                            wt = psp.tile([128, (SUBMAX // 128) * 16], bf16,
                                          tag="wt", name="wt")
                            for n in range(nch):
                                nc.tensor.transpose(
                                    out=wt[:, n * 16:(n + 1) * 16],
                                    in_=w16[0:16, n * 128:(n + 1) * 128],
                                    identity=identj_sb[0:16, :])
                            pend_wn.append((wn, wt, nch))
                            if len(pend_wn) > 1:
                                wn_, wt_, nch_ = pend_wn.pop(0)
                                nc.vector.tensor_copy(wn_[:], wt_[:, :nch_ * 16])
                        pend.append(dict(t=t, nch=nch, gn=gn, wn=wn, mk=mk,
                                         pw=pw, ci0=chunk_i,
                                         nct=n_chunks_tile,
                                         last=(si == n_sub - 1)))
                        chunk_i += nch
                        if len(pend) > PIPE_LAG:
                            flush_b()
                while pend_wn:
                    wn_, wt_, nch_ = pend_wn.pop(0)
                    nc.vector.tensor_copy(wn_[:], wt_[:, :nch_ * 16])
                while pend:
                    flush_b()

                # ---- GraphNorm stats (global) + h + skip
                s12 = sp.tile([128, 4], fp32, tag="s12", name="s12")  # [S1b0 S2b0 S1b1 S2b1]
                sq = sp.tile([128, NL], fp32, tag="sq", name="sq")
                for b in range(2):
                    nc.vector.tensor_reduce(
                        out=s12[:, 2 * b:2 * b + 1], in_=outT[b][:],
                        axis=mybir.AxisListType.X, op=mybir.AluOpType.add)
                    nc.vector.tensor_tensor(out=sq[:], in0=outT[b][:],
                                            in1=outT[b][:],
                                            op=mybir.AluOpType.mult)
                    nc.vector.tensor_reduce(
                        out=s12[:, 2 * b + 1:2 * b + 2], in_=sq[:],
                        axis=mybir.AxisListType.X, op=mybir.AluOpType.add)
                for b in range(2):
                    nc.sync.dma_start(out=st_in[b * 128:(b + 1) * 128, :],
                                      in_=s12[:, 2 * b:2 * b + 2])
                if do_coll:
                    nc.gpsimd.collective_compute(
                        "AllReduce", mybir.AluOpType.add,
                        replica_groups=groups, ins=[st_in[:]], outs=[st_out[:]])
                else:
                    nc.sync.dma_start(out=st_out[:], in_=st_in[:])
                s12g = sp.tile([128, 4], fp32, tag="s12g", name="s12g")
                for b in range(2):
                    nc.sync.dma_start(out=s12g[:, 2 * b:2 * b + 2],
                                      in_=st_out[b * 128:(b + 1) * 128, :])

                c1 = sp.tile([128, 2], fp32, tag="c1", name="c1")
                var = sp.tile([128, 2], fp32, tag="var", name="var")
                rstd = sp.tile([128, 2], fp32, tag="rstd", name="rstd")
                f_ = sp.tile([128, 2], fp32, tag="f_", name="f_")
                for b in range(2):
                    S1 = s12g[:, 2 * b:2 * b + 1]
                    S2 = s12g[:, 2 * b + 1:2 * b + 2]
                    # c1 = A + B*S1
                    nc.vector.tensor_tensor(out=c1[:, b:b + 1],
                                            in0=S1, in1=cbB_s[:, b:b + 1],
                                            op=mybir.AluOpType.mult)
                    nc.vector.tensor_tensor(out=c1[:, b:b + 1],
                                            in0=c1[:, b:b + 1],
                                            in1=cbA_s[:, b:b + 1],
                                            op=mybir.AluOpType.add)
                    # var = S2/N + c1*(2*S1/N + c1)
                    nc.vector.tensor_scalar(
                        out=var[:, b:b + 1], in0=S1, scalar1=2.0 / N,
                        scalar2=None, op0=mybir.AluOpType.mult)
                    nc.vector.tensor_tensor(out=var[:, b:b + 1],
                                            in0=var[:, b:b + 1],
                                            in1=c1[:, b:b + 1],
                                            op=mybir.AluOpType.add)
                    nc.vector.tensor_tensor(out=var[:, b:b + 1],
                                            in0=var[:, b:b + 1],
                                            in1=c1[:, b:b + 1],
                                            op=mybir.AluOpType.mult)
                    nc.vector.tensor_scalar(
                        out=var[:, b:b + 1], in0=S2, scalar1=1.0 / N,
                        scalar2=var[:, b:b + 1], op0=mybir.AluOpType.mult,
                        op1=mybir.AluOpType.add)
                    # rstd = 1/sqrt(var + eps)
                    nc.scalar.activation(
                        out=rstd[:, b:b + 1], in_=var[:, b:b + 1],
                        func=mybir.ActivationFunctionType.Sqrt, bias=eps_t[:])
                    nc.vector.reciprocal(rstd[:, b:b + 1], rstd[:, b:b + 1])
                    nc.vector.tensor_tensor(out=f_[:, b:b + 1],
                                            in0=rstd[:, b:b + 1],
                                            in1=gnw_s[:, b:b + 1],
                                            op=mybir.AluOpType.mult)
                    # h = (out + c1) * f + gnb  (into hT)
                    nc.vector.tensor_scalar(
                        out=hT[b][:], in0=outT[b][:],
                        scalar1=c1[:, b:b + 1], scalar2=None,
                        op0=mybir.AluOpType.add)
                    nc.vector.tensor_scalar(
                        out=hT[b][:], in0=hT[b][:],
                        scalar1=f_[:, b:b + 1], scalar2=gnb_s[:, b:b + 1],
                        op0=mybir.AluOpType.mult, op1=mybir.AluOpType.add)
                    # elu(h) - 1 fold: h' = relu(h) + exp(min(h,0))
                    nc.vector.tensor_scalar(
                        out=sq[:], in0=hT[b][:], scalar1=0.0, scalar2=None,
                        op0=mybir.AluOpType.min)
                    nc.scalar.activation(
                        out=sq[:], in_=sq[:],
                        func=mybir.ActivationFunctionType.Exp)
                    nc.vector.tensor_scalar(
                        out=hT[b][:], in0=hT[b][:], scalar1=0.0, scalar2=None,
                        op0=mybir.AluOpType.max)
                    nc.vector.tensor_tensor(
                        out=hT[b][:], in0=hT[b][:], in1=sq[:],
                        op=mybir.AluOpType.add)

                # skip matmul: xT += W_sk.T h' + bsk_eff
                for bo in range(2):
                    for ch0 in range(0, NL, 512):
                        cw = min(512, NL - ch0)
                        ps = psp.tile([128, 512], fp32, tag="wsum", name="wsum")
                        for bi in range(2):
                            nc.tensor.matmul(
                                out=ps[:, :cw],
                                lhsT=wsk_s[:, bi, bo],
                                rhs=hT[bi][:, ch0:ch0 + cw],
                                start=(bi == 0), stop=(bi == 1))
                        nc.vector.tensor_tensor(
                            out=xT[bo][:, ch0:ch0 + cw],
                            in0=xT[bo][:, ch0:ch0 + cw], in1=ps[:, :cw],
                            op=mybir.AluOpType.add)
                    nc.vector.tensor_scalar(
                        out=xT[bo][:], in0=xT[bo][:],
                        scalar1=bsk_s[:, bo:bo + 1], scalar2=None,
                        op0=mybir.AluOpType.add)
                    nc.vector.tensor_copy(xTb[bo][:], xT[bo][:])

            fcw_s = pp.tile([128, 2], bf16, tag="fcws", name="fcws")
            fcb_s = pp.tile([1, 1], fp32, tag="fcbs", name="fcbs")
            nc.sync.dma_start(out=fcw_s[:], in_=v_pb2("fcw_t"))
            nc.sync.dma_start(out=fcb_s[:], in_=fcb_ap)

            for l in range(n_layers):
                layer(l)

            # final fc
            y_sb = sp.tile([1, NL], fp32, tag="ysb", name="ysb")
            for ch0 in range(0, NL, 512):
                cw = min(512, NL - ch0)
                ps = psp.tile([128, 512], fp32, tag="lg", name="lg", bufs=3)
                for b in range(2):
                    nc.tensor.matmul(out=ps[:1, :cw], lhsT=fcw_s[:, b:b + 1],
                                     rhs=xTb[b][:, ch0:ch0 + cw],
                                     start=(b == 0), stop=(b == 1))
                nc.scalar.activation(
                    out=y_sb[:, ch0:ch0 + cw], in_=ps[:1, :cw],
                    func=mybir.ActivationFunctionType.Identity,
                    bias=fcb_s[:], scale=1.0)
            nc.sync.dma_start(out=y_out[:], in_=y_sb[:])

    nc.compile()
    return nc


# ---------------------------------------------------------------- runner

_CACHE = {}


def kernel(**inputs):
    layout, in_maps = prep_inputs(inputs)
    key = (layout["NL"], layout["E_pad"],
           tuple(int(x) for x in layout["tile_edge_base"]))
    if key not in _CACHE:
        _CACHE[key] = build_program(layout)
    nc = _CACHE[key]
    res = bass_utils.run_bass_kernel_spmd(nc, in_maps, core_ids=list(range(NCORES)))
    y = np.zeros(N, np.float32)
    for c in range(NCORES):
        yc = np.asarray(res.results[c]["y"], np.float32).reshape(-1)
        y[layout["core_nodes"][c]] = yc[layout["cores"][c]["slot_of_node"]]
    return y


if __name__ == "__main__":
    sys.path.insert(0, "/root/problem")
    import jax
    import reference

    with jax.default_device(jax.devices("cpu")[0]):
        inputs = {k: np.asarray(v) for k, v in reference.setup_inputs().items()}
        expected = np.asarray(reference.reference(**inputs))
    got = kernel(**inputs)
    rel = np.linalg.norm(got - expected) / np.linalg.norm(expected)
    print("rel l2:", rel)
    print(expected[:4], got[:4])



# revision 56
# speedup vs baseline: 1.0137x; 1.0137x over previous
"""Trainium2 Bass kernel for nn_EnhancedFinGAT (4-layer GATv2 + GraphNorm + skip).

Strategy (8 NeuronCores, SPMD):
  - Nodes (and their incoming edges) are sharded by destination across the 8
    cores; per-core nodes are permuted into degree-bucket-major "slots" so the
    per-edge xr[dst] add becomes a broadcast access pattern (no second gather).
  - Per layer: local matmuls produce xl (normal layout, AllGathered into a
    DRAM table) and xr (transposed, SBUF).  Edge phase per 128-dst tile:
    one transposed dma_gather (for PE logits) + one normal dma_gather (for the
    PE mask-matmul weighted segment sum), leaky-relu via the |z| decomposition,
    softmax without max-subtraction (logits are bounded), exp on ScalarE,
    per-128-edge mask matmuls accumulate both the weighted sums and the
    softmax denominators in PSUM.
  - GraphNorm stats via one AllReduce of (sum, sum-of-squares); skip matmul in
    transposed layout.
All heavy per-edge data is bf16; accumulations are f32.
"""

import os
import sys
import numpy as np

sys.path.insert(0, "/opt/trn_rl_repo")

import concourse.bass as bass
import concourse.bacc as bacc
import concourse.mybir as mybir
import concourse.tile as tile
import concourse.bass_utils as bass_utils
from concourse.masks import make_identity

fp32 = mybir.dt.float32
f32r = mybir.dt.float32r
bf16 = mybir.dt.bfloat16
i16 = mybir.dt.int16

N, HID, L, H, CH = 10000, 256, 4, 4, 64
NCORES = 8
NPC = N // NCORES
EPS = 1e-5
P = 128


# ---------------------------------------------------------------- host prep

def _bucket_of(deg):
    if deg <= 128:
        return max(2, ((deg + 1) // 2) * 2)
    for k in (136, 144, 152, 160, 176, 192, 208, 224, 240, 256, 320, 384, 512):
        if deg <= k:
            return k
    raise ValueError(deg)


def _cumcount(x):
    n = len(x)
    if n == 0:
        return np.zeros(0, np.int64)
    change = np.empty(n, dtype=bool)
    change[0] = True
    change[1:] = x[1:] != x[:-1]
    run_starts = np.flatnonzero(change)
    return np.arange(n) - run_starts[np.cumsum(change) - 1]


def build_layout(edge_index, sub_cap=2048):
    src_g = np.concatenate([np.asarray(edge_index[0], np.int64), np.arange(N)])
    dst_g = np.concatenate([np.asarray(edge_index[1], np.int64), np.arange(N)])

    # degree-balanced node->core assignment: deal nodes (sorted by degree)
    # round-robin so the per-core bucket histograms almost coincide.
    deg_g = np.bincount(dst_g, minlength=N)
    order = np.argsort(-deg_g, kind="stable")
    core_nodes = [np.sort(order[c::NCORES]) for c in range(NCORES)]
    loc_of = np.zeros(N, np.int64)
    core_of_node = np.zeros(N, np.int64)
    for c in range(NCORES):
        loc_of[core_nodes[c]] = np.arange(NPC)
        core_of_node[core_nodes[c]] = c
    core_of = core_of_node[dst_g]

    per_core = []
    all_buckets = {}
    for c in range(NCORES):
        m = core_of == c
        s, d = src_g[m], loc_of[dst_g[m]]
        deg = np.bincount(d, minlength=NPC)
        buckets = np.array([_bucket_of(x) for x in deg])
        cnt = {}
        for k in buckets:
            cnt[int(k)] = cnt.get(int(k), 0) + 1
        for k, v in cnt.items():
            all_buckets[k] = max(all_buckets.get(k, 0), v)
        per_core.append((s, d, buckets))

    ks = sorted(all_buckets)
    M = {k: all_buckets[k] for k in ks}
    NL = ((sum(M.values()) + P - 1) // P) * P
    NT = NCORES * NL
    PAD_ROW = 0

    slot_bucket = np.zeros(NL, np.int64)
    off = 0
    bucket_slot_base = {}
    for k in ks:
        bucket_slot_base[k] = off
        slot_bucket[off:off + M[k]] = k
        off += M[k]

    n_tiles = NL // P
    SUB_CAP = sub_cap
    slot_edge_off = np.zeros(NL, np.int64)
    tile_edge_base = np.zeros(n_tiles + 1, np.int64)
    tile_subs = []  # per tile: list of (e_start, e_end, [(k, d0, m), ...])
    e = 0
    for t in range(n_tiles):
        tile_edge_base[t] = e
        subs = []
        sub_start = e
        sub_runs = []
        run = None  # (k, d0, m)
        for d in range(t * P, (t + 1) * P):
            k = int(slot_bucket[d])
            if k == 0:
                continue
            pad_now = ((e + P - 1) // P) * P
            if pad_now + k - sub_start > SUB_CAP:
                # close current sub before this node
                if run is not None:
                    sub_runs.append(run)
                    run = None
                e = pad_now
                subs.append((int(sub_start), int(e), sub_runs))
                sub_runs = []
                sub_start = e
            slot_edge_off[d] = e
            if run is not None and run[0] == k:
                run = (k, run[1], run[2] + 1)
            else:
                if run is not None:
                    sub_runs.append(run)
                run = (k, d, 1)
            e += k
        if run is not None:
            sub_runs.append(run)
        if e > sub_start or sub_runs:
            e = ((e + P - 1) // P) * P
            subs.append((int(sub_start), int(e), sub_runs))
        tile_subs.append(subs)
    tile_edge_base[n_tiles] = e
    E_pad = int(e)

    cores = []
    for c in range(NCORES):
        s, d, buckets = per_core[c]
        slot_of_node = np.full(NPC, -1, np.int64)
        next_free = dict(bucket_slot_base)
        for n_loc in np.argsort(buckets, kind="stable"):
            k = int(buckets[n_loc])
            slot_of_node[n_loc] = next_free[k]
            next_free[k] += 1
        order = np.argsort(slot_of_node[d], kind="stable")
        cores.append(dict(slot_of_node=slot_of_node,
                          s_sorted=s[order],
                          d_sorted_slot=slot_of_node[d][order]))

    g2p = np.zeros(N, np.int64)
    for c in range(NCORES):
        g2p[core_nodes[c]] = c * NL + cores[c]["slot_of_node"]

    # run membership arrays: edges of a run (k, d0, m) are interleaved
    # dst-major (edge j of dst i at run_base + j*m + i) so the xr[dst]
    # broadcast add has a packed last dim (DVE 2x mode).
    run_d0 = np.arange(NL)
    run_m = np.ones(NL, np.int64)
    for subs in tile_subs:
        for (_e0, _e1, runs) in subs:
            for (k, d0, m) in runs:
                run_d0[d0:d0 + m] = d0
                run_m[d0:d0 + m] = m

    for c in range(NCORES):
        src_slot = np.full(E_pad, PAD_ROW, np.int64)
        dst_slot = np.full(E_pad, -1, np.int64)
        d_sl = cores[c]["d_sorted_slot"]
        pos = (slot_edge_off[run_d0[d_sl]] + _cumcount(d_sl) * run_m[d_sl]
               + (d_sl - run_d0[d_sl]))
        src_slot[pos] = g2p[cores[c]["s_sorted"]]
        dst_slot[pos] = d_sl
        cores[c]["src_slot"] = src_slot
        cores[c]["dst_slot_of_edge"] = dst_slot

    return dict(NL=int(NL), NT=int(NT), PAD_ROW=int(PAD_ROW), E_pad=E_pad,
                n_tiles=n_tiles, tile_edge_base=tile_edge_base,
                tile_subs=tile_subs, slot_edge_off=slot_edge_off,
                core_nodes=core_nodes, g2p=g2p, sub_cap=int(SUB_CAP)), cores


def wrap_idx16(idx):
    n = len(idx)
    cols = (n + 15) // 16
    pad = np.zeros(cols * 16, np.int64)
    pad[:n] = idx
    w = pad.reshape(cols, 16).T.astype(np.int16)
    return np.tile(w, (8, 1))


def build_masks(layout, core):
    E_pad = layout["E_pad"]
    n_chunks = E_pad // P
    dst = core["dst_slot_of_edge"]
    masks = np.zeros((n_chunks, P, P), np.float32)
    for chn in range(n_chunks):
        d = dst[chn * P:(chn + 1) * P]
        rows = np.flatnonzero(d >= 0)
        masks[chn, rows, (d[d >= 0] % P)] = 1.0
    return masks


def att4_lhst(att_l, scale):
    # features are channel-interleaved: feature f holds (h, c) = (f%4, f//4)
    out = np.zeros((2, P, P), np.float32)
    for b in range(2):
        for p in range(P):
            f = 128 * b + p
            h = f % H
            out[b, p, h::4] = scale * att_l[h, f // H]
    return out


def _to_bf16(x):
    import jax.numpy as jnp
    return np.asarray(jnp.asarray(x, jnp.bfloat16)).view(np.uint16)


# numpy bf16 arrays are passed as uint16 views?  Simpler: use ml_dtypes.
def to_bf16(x):
    import ml_dtypes
    return np.asarray(x, np.float32).astype(ml_dtypes.bfloat16)


def prep_inputs(inputs, sub_cap=2048):
    """Returns (layout, in_maps) — one dict per core."""
    layout, cores = build_layout(np.asarray(inputs["edge_index"]), sub_cap=sub_cap)
    NL, E_pad = layout["NL"], layout["E_pad"]

    x = np.asarray(inputs["x"], np.float32)
    lw = np.asarray(inputs["lin_l_w"], np.float32)
    lb = np.asarray(inputs["lin_l_b"], np.float32)
    rw = np.asarray(inputs["lin_r_w"], np.float32)
    rb = np.asarray(inputs["lin_r_b"], np.float32)
    att = np.asarray(inputs["att"], np.float32)
    cb = np.asarray(inputs["conv_bias"], np.float32)
    gnw = np.asarray(inputs["gn_weight"], np.float32)
    gnb = np.asarray(inputs["gn_bias"], np.float32)
    gnm = np.asarray(inputs["gn_mean_scale"], np.float32)
    skw = np.asarray(inputs["skip_w"], np.float32)

    # channel-interleaved h-space: new feature c*H+h <- old h*CH+c
    PERM = np.zeros(HID, np.int64)
    for h in range(H):
        PERM[np.arange(CH) * H + h] = h * CH + np.arange(CH)
    lw = lw[:, :, PERM]
    lb = lb[:, PERM]
    rw = rw[:, :, PERM]
    rb = rb[:, PERM]
    cb = cb[:, PERM]
    gnw = gnw[:, PERM]
    gnb = gnb[:, PERM]
    gnm = gnm[:, PERM]
    skw = skw[PERM, :]
    skb = np.asarray(inputs["skip_b"], np.float32)
    fcw = np.asarray(inputs["fc_w"], np.float32)
    fcb = np.asarray(inputs["fc_b"], np.float32)

    # layer-stacked common weights (same on all cores)
    wl_in = lw.reshape(L, 2, 128, 256)                    # [l, bi, 128, 256]
    wr_in = rw.reshape(L, 2, 128, 256)
    wsk_in = skw.reshape(2, 128, 2, 128).transpose(0, 2, 1, 3)  # [bi, bo, 128, 128]
    bl_in = lb.reshape(L, 1, 256)
    br_t = rb.reshape(L, 2, 128, 1)
    bsk_eff = (skb - skw.sum(axis=0)).reshape(2, 128, 1)
    a4z = np.stack([att4_lhst(att[l], 0.6) for l in range(L)])  # [L, 2, 128, 128]
    a4a = np.stack([att4_lhst(att[l], 0.4) for l in range(L)])
    cbA = (cb * (1.0 - gnm)).reshape(L, 2, 128, 1)
    cbB = np.broadcast_to((-gnm / float(N)).reshape(L, 2, 128, 1), (L, 2, 128, 1))
    gnw_t = gnw.reshape(L, 2, 128, 1)
    gnb_t = gnb.reshape(L, 2, 128, 1)
    fcw_t = fcw.reshape(2, 128, 1)
    fcb_in = fcb.reshape(1, 1)

    identj = np.zeros((128, 16), np.float32)
    for j in range(3):
        identj[32 * j:32 * j + 16] = np.eye(16)

    common = dict(
        wl=to_bf16(wl_in), wr=to_bf16(wr_in), wsk=to_bf16(wsk_in.copy()),
        bl=to_bf16(bl_in), br_t=br_t,
        bsk=bsk_eff, a4z=to_bf16(a4z), a4a=to_bf16(a4a),
        cbA=cbA, cbB=np.asarray(cbB, np.float32).copy(), gnw_t=gnw_t, gnb_t=gnb_t,
        fcw_t=to_bf16(fcw_t), fcb=fcb_in, identj=to_bf16(identj),
    )

    in_maps = []
    for c in range(NCORES):
        core = cores[c]
        x0 = np.zeros((NL, HID), np.float32)
        x0[core["slot_of_node"]] = x[layout["core_nodes"][c]]
        x0t = x0.T.reshape(2, 128, NL).copy()
        m = dict(common)
        m["x0t"] = x0t
        m["iotaw"] = wrap_idx16(np.arange(layout["sub_cap"]))
        m["srcw"] = wrap_idx16(core["src_slot"])
        d2 = core["dst_slot_of_edge"].reshape(-1, 128).T
        m["dsts"] = np.where(d2 >= 0, d2 % 128, -1).astype(np.float32)
        in_maps.append(pack_core(layout, m))

    layout["cores"] = cores
    return layout, in_maps


def pack_spec(layout):
    """Fixed packing of all external inputs into 3 dtype buffers.
    Returns name -> (buf_key, offset, shape); buf keys: pb (bf16), pf (f32),
    pi (i16)."""
    NL, E_pad = layout["NL"], layout["E_pad"]
    EW = (E_pad + 15) // 16
    NCH = E_pad // P
    sc = layout["sub_cap"]
    specs = {}
    offs = {"pb": 0, "pf": 0, "pi": 0}

    def add(key, name, shape):
        specs[name] = (key, offs[key], tuple(shape))
        offs[key] += int(np.prod(shape))

    add("pb", "wl", (L, 2, 128, 256))
    add("pb", "wr", (L, 2, 128, 256))
    add("pb", "wsk", (2, 2, 128, 128))
    add("pb", "bl", (L, 1, 256))
    add("pb", "a4z", (L, 2, 128, 128))
    add("pb", "a4a", (L, 2, 128, 128))
    add("pb", "fcw_t", (2, 128, 1))
    add("pb", "identj", (128, 16))
    add("pf", "x0t", (2, 128, NL))
    add("pf", "br_t", (L, 2, 128, 1))
    add("pf", "bsk", (2, 128, 1))
    add("pf", "cbA", (L, 2, 128, 1))
    add("pf", "cbB", (L, 2, 128, 1))
    add("pf", "gnw_t", (L, 2, 128, 1))
    add("pf", "gnb_t", (L, 2, 128, 1))
    add("pf", "fcb", (1, 1))
    add("pf", "dsts", (128, NCH))
    add("pi", "srcw", (128, EW))
    add("pi", "iotaw", (128, sc // 16))
    return specs, dict(offs)


def pack_core(layout, tensors):
    import ml_dtypes
    specs, sizes = pack_spec(layout)
    out = {
        "pb": np.zeros((1, sizes["pb"]), ml_dtypes.bfloat16),
        "pf": np.zeros((1, sizes["pf"]), np.float32),
        "pi": np.zeros((1, sizes["pi"]), np.int16),
    }
    for name, (key, off, shape) in specs.items():
        arr = tensors[name]
        assert tuple(arr.shape) == tuple(shape), (name, arr.shape, shape)
        out[key][0, off:off + arr.size] = np.asarray(arr).reshape(-1)
    return out


# ---------------------------------------------------------------- device build

def build_program(layout, n_layers=L, do_edges=True, do_coll=True, edge_stage=5,
                  do_gt=True, do_gn=True, do_mk=True, spk=False, gt_from_sbuf=False,
                  gt_from_table=False, alt_queues=False, split_q=False, gn_from_gt=False,
                  dma_scratch=16384):
    NL, NT, E_pad = layout["NL"], layout["NT"], layout["E_pad"]
    n_tiles = layout["n_tiles"]
    teb = layout["tile_edge_base"]
    EW = (E_pad + 15) // 16
    NCH = E_pad // P
    SUBMAX = 128 * max(
        (s[1] - s[0]) // 128 for subs in layout["tile_subs"] for s in subs)

    nc = bacc.Bacc("TRN2", target_bir_lowering=False, debug=False,
                   num_devices=NCORES, num_swdge_queues=4 if (alt_queues or split_q) else 2,
                   dynamic_dma_scratch_size=dma_scratch)

    # ---- I/O: all external inputs packed into 3 dtype buffers
    specs, sizes = pack_spec(layout)
    pb = nc.dram_tensor("pb", [1, sizes["pb"]], bf16, kind="ExternalInput")
    pf = nc.dram_tensor("pf", [1, sizes["pf"]], fp32, kind="ExternalInput")
    pi = nc.dram_tensor("pi", [1, sizes["pi"]], i16, kind="ExternalInput")
    bufs = {"pb": pb, "pf": pf, "pi": pi}

    def pslice(name, idx=None):
        """Flat [1, numel] AP of tensor `name`, optionally of slice [idx]
        along dim 0."""
        key, off, shape = specs[name]
        numel = int(np.prod(shape))
        if idx is None:
            return bufs[key][0:1, off:off + numel]
        sub = numel // shape[0]
        return bufs[key][0:1, off + idx * sub:off + (idx + 1) * sub]

    def v_pbc(name, idx, b, p_, c):      # [l] . "b p c -> p b c"
        return pslice(name, idx).rearrange("o (b p c) -> (o p) b c", b=b, p=p_, c=c)

    def v_pb2(name, idx=None):           # [l] . "b p o -> p (b o)" (o==1)
        return pslice(name, idx).rearrange("z (b p) -> (z p) b", b=2, p=128)

    x0t_ap = lambda b: pslice("x0t", b).rearrange("o (p n) -> (o p) n", p=128)
    srcw_ap = pslice("srcw").rearrange("o (p e) -> (o p) e", p=128)
    iotaw_ap = pslice("iotaw").rearrange("o (p e) -> (o p) e", p=128)
    dsts_ap = pslice("dsts").rearrange("o (p n) -> (o p) n", p=128, n=NCH)
    identj_ap = pslice("identj").rearrange("o (p c) -> (o p) c", p=128)
    fcb_ap = pslice("fcb")

    y_out = nc.dram_tensor("y", [1, NL], fp32, kind="ExternalOutput")

    # ---- internal DRAM
    xl_bounce = nc.dram_tensor("xl_bounce", [NL, 256], bf16, kind="Internal")
    masks_dram = nc.dram_tensor("masks_dram", [128, NCH, 128], bf16, kind="Internal")
    st_in = nc.dram_tensor("st_in", [256, 2], fp32, kind="Internal")
    st_out = nc.dram_tensor("st_out", [256, 2], fp32, kind="Internal",
                            addr_space="Shared")
    xlt_sh = nc.dram_tensor("xlt_sh", [NT, 256], bf16, kind="Internal",
                            addr_space="Shared")

    groups = [list(range(NCORES))]

    with tile.TileContext(nc) as tc:
        with tc.tile_pool(name="persist", bufs=1) as pp, \
             tc.tile_pool(name="work", bufs=2) as wp, \
             tc.tile_pool(name="single", bufs=1) as sp, \
             tc.tile_pool(name="psum", bufs=2, space="PSUM") as psp:

            # ---------------- constants / persistent state
            ident128 = pp.tile([128, 128], fp32, tag="id128", name="id128")
            make_identity(nc, ident128[:])
            ident128b = pp.tile([128, 128], bf16, tag="id128b", name="id128b")
            nc.vector.tensor_copy(ident128b[:], ident128[:])
            identj_sb = pp.tile([128, 16], bf16, tag="idj", name="idj")
            nc.sync.dma_start(out=identj_sb[:], in_=identj_ap)

            ones_row = pp.tile([1, 128], bf16, tag="ones", name="ones")
            eps_t = pp.tile([128, 1], fp32, tag="epsT", name="epsT")
            nc.gpsimd.memset(eps_t[:], EPS)
            nc.gpsimd.memset(ones_row[:], 1.0)

            xT = [pp.tile([128, NL], fp32, tag=f"xT{b}", name=f"xT{b}") for b in range(2)]
            xrT = [pp.tile([128, NL], bf16, tag=f"xrT{b}", name=f"xrT{b}") for b in range(2)]
            xrTf = [pp.tile([128, NL], fp32, tag=f"xrTf{b}", name=f"xrTf{b}") for b in range(2)]
            xTb = [pp.tile([128, NL], bf16, tag=f"xTb{b}", name=f"xTb{b}") for b in range(2)]
            outT = [pp.tile([128, NL], fp32, tag=f"outT{b}", name=f"outT{b}") for b in range(2)]
            hT = [pp.tile([128, NL], bf16, tag=f"hT{b}", name=f"hT{b}") for b in range(2)]
            xl_sb = pp.tile([128, (NL // 128) * 256], bf16, tag="xlsb", name="xlsb")
            xlt_sb = (pp.tile([128, NT // 128, 256], bf16, tag="xltsb", name="xltsb")
                      if gt_from_table else None)
            srcw_sb = pp.tile([128, EW], i16, tag="srcsb", name="srcsb")
            nc.sync.dma_start(out=srcw_sb[:], in_=srcw_ap)
            iotaw_sb = pp.tile([128, layout["sub_cap"] // 16], i16, tag="iotasb", name="iotasb")
            nc.sync.dma_start(out=iotaw_sb[:], in_=iotaw_ap)
            dsts_sb = pp.tile([128, NCH], fp32, tag="dstssb", name="dstssb")
            nc.sync.dma_start(out=dsts_sb[:], in_=dsts_ap)
            iota_d = pp.tile([128, 128], fp32, tag="iotad", name="iotad")
            nc.gpsimd.iota(iota_d[:], pattern=[[1, 128]], base=0,
                           channel_multiplier=0,
                           allow_small_or_imprecise_dtypes=True)

            for b in range(2):
                nc.sync.dma_start(out=xT[b][:], in_=x0t_ap(b))
                nc.vector.tensor_copy(xTb[b][:], xT[b][:])

            # per-layer weight staging
            wl_s = pp.tile([128, 2, 256], bf16, tag="wls", name="wls")
            wr_s = pp.tile([128, 2, 256], bf16, tag="wrs", name="wrs")
            wsk_s = pp.tile([128, 2, 2, 128], bf16, tag="wsks", name="wsks")
            bl_s = pp.tile([1, 256], bf16, tag="bls", name="bls")
            br_s = pp.tile([128, 2], fp32, tag="brs", name="brs")
            bsk_s = pp.tile([128, 2], fp32, tag="bsks", name="bsks")
            a4z_s = pp.tile([128, 2, 128], bf16, tag="a4zs", name="a4zs")
            a4a_s = pp.tile([128, 2, 128], bf16, tag="a4as", name="a4as")
            cbA_s = pp.tile([128, 2], fp32, tag="cbAs", name="cbAs")
            cbB_s = pp.tile([128, 2], fp32, tag="cbBs", name="cbBs")
            gnw_s = pp.tile([128, 2], fp32, tag="gnws", name="gnws")
            gnb_s = pp.tile([128, 2], fp32, tag="gnbs", name="gnbs")

            for b in range(2):
                nc.sync.dma_start(out=wsk_s[:, b], in_=pslice("wsk", b).rearrange("z (o p c) -> (z p) o c", o=2, p=128, c=128))
            nc.sync.dma_start(out=bsk_s[:], in_=v_pb2("bsk"))

            def layer(l):
                # ---- stage layer weights
                nc.sync.dma_start(out=wl_s[:], in_=v_pbc("wl", l, 2, 128, 256))
                nc.sync.dma_start(out=wr_s[:], in_=v_pbc("wr", l, 2, 128, 256))
                nc.sync.dma_start(out=bl_s[:], in_=pslice("bl", l))
                nc.sync.dma_start(out=br_s[:], in_=v_pb2("br_t", l))
                nc.sync.dma_start(out=a4z_s[:], in_=v_pbc("a4z", l, 2, 128, 128))
                nc.sync.dma_start(out=a4a_s[:], in_=v_pbc("a4a", l, 2, 128, 128))
                nc.sync.dma_start(out=cbA_s[:], in_=v_pb2("cbA", l))
                nc.sync.dma_start(out=cbB_s[:], in_=v_pb2("cbB", l))
                nc.sync.dma_start(out=gnw_s[:], in_=v_pb2("gnw_t", l))
                nc.sync.dma_start(out=gnb_s[:], in_=v_pb2("gnb_t", l))

                # ---- xl (normal layout) and xr (transposed) from x_T
                for t in range(NL // 128):
                    ps = psp.tile([128, 512], fp32, tag="lg", name="lg", bufs=3)
                    for bi in range(2):
                        nc.tensor.matmul(out=ps[:, :256],
                                         lhsT=xTb[bi][:, t * 128:(t + 1) * 128],
                                         rhs=wl_s[:, bi],
                                         start=(bi == 0), stop=False)
                    nc.tensor.matmul(out=ps[:, :256], lhsT=ones_row[:],
                                     rhs=bl_s[:], start=False, stop=True)
                    nc.vector.tensor_copy(
                        xl_sb[:, t * 256:(t + 1) * 256], ps[:, :256])
                nc.sync.dma_start(
                    out=xl_bounce[:].rearrange("(t p) c -> p t c", p=128),
                    in_=xl_sb[:].rearrange("p (t c) -> p t c", c=256))

                # xr transposed: out block bo over node chunks of 512
                for bo in range(2):
                    for ch0 in range(0, NL, 512):
                        cw = min(512, NL - ch0)
                        ps = psp.tile([128, 512], fp32, tag="wsum", name="wsum")
                        for bi in range(2):
                            nc.tensor.matmul(
                                out=ps[:, :cw],
                                lhsT=wr_s[:, bi, bo * 128:(bo + 1) * 128],
                                rhs=xTb[bi][:, ch0:ch0 + cw],
                                start=(bi == 0), stop=(bi == 1))
                        nc.scalar.activation(
                            out=xrT[bo][:, ch0:ch0 + cw], in_=ps[:, :cw],
                            func=mybir.ActivationFunctionType.Identity,
                            bias=br_s[:, bo:bo + 1], scale=1.0)
                        nc.scalar.activation(
                            out=xrTf[bo][:, ch0:ch0 + cw], in_=ps[:, :cw],
                            func=mybir.ActivationFunctionType.Identity,
                            bias=br_s[:, bo:bo + 1], scale=1.0)

                # ---- AllGather xl into the table
                if do_coll:
                    nc.gpsimd.collective_compute(
                        "AllGather", mybir.AluOpType.bypass,
                        replica_groups=groups,
                        ins=[xl_bounce[:]],
                        outs=[xlt_sh[:]],
                    )
                if gt_from_table:
                    nc.sync.dma_start(
                        out=xlt_sb[:],
                        in_=xlt_sh[:].rearrange("(t p) c -> p t c", p=128))

                # ---- edge phase, per dst-tile, sub-chunked
                sub_seq = [0]
                cpi = [0]
                if not do_edges or edge_stage < 5:
                    for b in range(2):
                        nc.gpsimd.memset(outT[b][:], 0.0)
                # software-pipelined emission: defer each sub's B-phase
                # (wg + mask matmuls + tile-post) by PIPE_LAG subs so in-order
                # engines don't stall on the long gt->wn chain.
                PIPE_LAG = 3
                pend = []
                pend_wn = []

                def flush_b():
                    d = pend.pop(0)
                    nch_ = d["nch"]
                    pw_ = d["pw"]
                    if edge_stage >= 4:
                        wg = wp.tile([128, nch_, 260], bf16, tag="wg", name="wg",
                                     padded_shape=[128, SUBMAX // 128, 260],
                                     bufs=3)
                        nc.vector.tensor_tensor(
                            out=wg[:, :, 0:256].rearrange("p n (c f) -> p n c f", f=4),
                            in0=d["gn"][:].rearrange("p n (c f) -> p n c f", f=4),
                            in1=d["wn"][:, :, 0:4].to_broadcast([128, nch_, 4, 64])
                                .rearrange("p n f c -> p n c f"),
                            op=mybir.AluOpType.mult)
                        nc.vector.tensor_copy(wg[:, :, 256:260], d["wn"][:, :, 0:4])
                        for n in range(nch_ if edge_stage >= 5 else 0):
                            nc.tensor.matmul(out=pw_[:, :260], lhsT=d["mk"][:, n],
                                             rhs=wg[:, n],
                                             start=(d["ci0"] + n == 0),
                                             stop=(d["ci0"] + n == d["nct"] - 1))
                    if d["last"] and edge_stage >= 5:
                        t_ = d["t"]
                        srec = wp.tile([128, 4], fp32, tag="srec", name="srec")
                        nc.vector.tensor_scalar(
                            out=srec[:], in0=pw_[:, 256:260], scalar1=1e-20,
                            scalar2=None, op0=mybir.AluOpType.add)
                        nc.vector.reciprocal(srec[:], srec[:])
                        outn = wp.tile([128, 256], fp32, tag="outn", name="outn")
                        nc.vector.tensor_tensor(
                            out=outn[:].rearrange("p (c f) -> p c f", f=4),
                            in0=pw_[:, 0:256].rearrange("p (c f) -> p c f", f=4),
                            in1=srec[:].to_broadcast([128, 4, 64])
                                .rearrange("p f c -> p c f"),
                            op=mybir.AluOpType.mult)
                        for b in range(2):
                            tp = psp.tile([128, 128], fp32, tag="tpo", name="tpo", bufs=1)
                            nc.tensor.transpose(
                                out=tp[:], in_=outn[:, b * 128:(b + 1) * 128],
                                identity=ident128[:])
                            nc.vector.tensor_copy(
                                outT[b][:, t_ * 128:(t_ + 1) * 128], tp[:])

                for t in range(n_tiles if do_edges else 0):
                    subs = layout["tile_subs"][t]
                    n_sub = len(subs)
                    chunk_i = 0
                    n_chunks_tile = sum((s[1] - s[0]) // 128 for s in subs)
                    for si, (e0, e1, runs) in enumerate(subs):
                        et = e1 - e0
                        nch = et // 128
                        if si == 0:
                            pw = psp.tile([128, 512], fp32, tag="wsum", name="wsum")
                        gbufs = {512: 6, 1024: 4, 2048: 3}.get(layout["sub_cap"], 2)
                        gt = wp.tile([128, 2 * et], bf16, tag="gt", name="gt",
                                     padded_shape=[128, 2 * SUBMAX], bufs=gbufs)
                        gn = wp.tile([128, nch, 256], bf16, tag="gn", name="gn",
                                     padded_shape=[128, SUBMAX // 128, 256],
                                     bufs=gbufs + 1)
                        mk = wp.tile([128, nch, 128], bf16, tag="mk", name="mk",
                                     padded_shape=[128, SUBMAX // 128, 128],
                                     bufs=gbufs + 1)
                        za = wp.tile([128, 2 * et], bf16, tag="za", name="za",
                                     padded_shape=[128, 2 * SUBMAX], bufs=2)
                        w16 = wp.tile([16, et], bf16, tag="w16", name="w16",
                                      padded_shape=[16, SUBMAX], bufs=gbufs + 1)
                        wn = wp.tile([128, nch, 16], bf16, tag="wn", name="wn",
                                     padded_shape=[128, SUBMAX // 128, 16],
                                     bufs=gbufs + 1)

                        qbase = (sub_seq[0] % 2) * 2 if alt_queues else 0
                        sub_seq[0] += 1
                        if do_gt and not gt_from_sbuf and not gt_from_table:
                            if split_q:
                                for b in range(2):
                                    nc.gpsimd.dma_gather(
                                        out_ap=gt[:, b * et:(b + 1) * et].rearrange("p (u e) -> p u e", u=1),
                                        in_ap=xlt_sh[:, b * 128:(b + 1) * 128],
                                        idxs_ap=srcw_sb[:, e0 // 16:e1 // 16],
                                        num_idxs=et, num_idxs_reg=et,
                                        elem_size=128, elem_step=256,
                                        transpose=True, single_packet=spk,
                                        queue_num=2 * b)
                            else:
                                nc.gpsimd.dma_gather(
                                    out_ap=gt[:].rearrange("p (b e) -> p b e", b=2), in_ap=xlt_sh[:],
                                    idxs_ap=srcw_sb[:, e0 // 16:e1 // 16],
                                    num_idxs=et, num_idxs_reg=et, elem_size=256,
                                    transpose=True, single_packet=spk,
                                    queue_num=qbase)
                        if do_gt and gt_from_table:
                            nc.gpsimd.dma_gather(
                                out_ap=gt[:].rearrange("p (b e) -> p b e", b=2),
                                in_ap=xlt_sb[:],
                                idxs_ap=srcw_sb[:, e0 // 16:e1 // 16],
                                num_idxs=et, num_idxs_reg=et, elem_size=256,
                                transpose=True, single_packet=spk,
                                sbuf_tokens_per_rank=128,
                                sbuf_free_dim_per_rank=512,
                                sbuf_free_dim_pad_per_rank=0,
                                sbuf_byte_offset=0)
                        if do_gt and gt_from_sbuf:
                            nc.gpsimd.dma_gather(
                                out_ap=gt[:].rearrange("p (b e) -> p b e", b=2),
                                in_ap=gn[:],
                                idxs_ap=iotaw_sb[:, :et // 16],
                                num_idxs=et, num_idxs_reg=et, elem_size=256,
                                transpose=True, single_packet=spk,
                                sbuf_tokens_per_rank=128,
                                sbuf_free_dim_per_rank=512,
                                sbuf_free_dim_pad_per_rank=0,
                                sbuf_byte_offset=0)
                        if do_gn and gn_from_gt:
                            for g4 in range(0, nch, 4):
                                gw4 = min(4, nch - g4)
                                for b in range(2):
                                    tpg = psp.tile([128, 512], bf16, tag="tpg",
                                                   name="tpg")
                                    for k in range(gw4):
                                        n = g4 + k
                                        nc.tensor.transpose(
                                            out=tpg[:, k * 128:(k + 1) * 128],
                                            in_=gt[:, b * et + n * 128:
                                                   b * et + (n + 1) * 128],
                                            identity=ident128b[:])
                                    dst_ap = gn[:, g4:g4 + gw4,
                                                b * 128:(b + 1) * 128]
                                    src_ap = tpg[:, :gw4 * 128].rearrange(
                                        "p (k c) -> p k c", k=gw4)
                                    if cpi[0] % 2 == 0:
                                        nc.vector.tensor_copy(dst_ap, src_ap)
                                    else:
                                        nc.scalar.activation(
                                            out=dst_ap, in_=src_ap,
                                            func=mybir.ActivationFunctionType.Copy)
                                    cpi[0] += 1
                        if do_gn and not gn_from_gt:
                            if split_q and et >= 256:
                                eh0 = (nch // 2) * 128
                                for (s0, s1, q) in ((0, eh0, 1), (eh0, et, 3)):
                                    nc.gpsimd.dma_gather(
                                        out_ap=gn[:, s0 // 128:s1 // 128],
                                        in_ap=xlt_sh[:],
                                        idxs_ap=srcw_sb[:, (e0 + s0) // 16:(e0 + s1) // 16],
                                        num_idxs=s1 - s0, num_idxs_reg=s1 - s0,
                                        elem_size=256,
                                        transpose=False, single_packet=spk,
                                        queue_num=q)
                            else:
                                nc.gpsimd.dma_gather(
                                    out_ap=gn[:], in_ap=xlt_sh[:],
                                    idxs_ap=srcw_sb[:, e0 // 16:e1 // 16],
                                    num_idxs=et, num_idxs_reg=et, elem_size=256,
                                    transpose=False, single_packet=spk,
                                    queue_num=qbase + 1)
                        if do_mk:
                            if l == 0:
                                nc.vector.tensor_tensor(
                                    out=mk[:],
                                    in0=dsts_sb[:, e0 // 128:e1 // 128]
                                        .to_broadcast([128, nch, 128]),
                                    in1=iota_d[:].rearrange("p (u c) -> p u c", u=1)
                                        .to_broadcast([128, nch, 128]),
                                    op=mybir.AluOpType.is_equal)
                                nc.sync.dma_start(
                                    out=masks_dram[:, e0 // 128:e1 // 128, :],
                                    in_=mk[:])
                            else:
                                nc.sync.dma_start(
                                    out=mk[:],
                                    in_=masks_dram[:, e0 // 128:e1 // 128, :])

                        # z = g + xr[dst] via bucket-broadcast; pad tail stays g
                        for b in range(2 if edge_stage >= 2 else 0):
                            for (k, d0, m) in runs:
                                off = int(layout["slot_edge_off"][d0] - e0)
                                base = b * et + off
                                if m == 1:
                                    nc.vector.tensor_scalar(
                                        out=gt[:, base:base + k],
                                        in0=gt[:, base:base + k],
                                        scalar1=xrTf[b][:, d0:d0 + 1],
                                        scalar2=None, op0=mybir.AluOpType.add)
                                else:
                                    seg = gt[:, base:base + m * k] \
                                        .rearrange("p (k m) -> p k m", m=m)
                                    nc.vector.tensor_tensor(
                                        out=seg, in0=seg,
                                        in1=xrT[b][:, d0:d0 + m]
                                            .to_broadcast([128, m, k])
                                            .rearrange("p m k -> p k m"),
                                        op=mybir.AluOpType.add)
                            nc.scalar.activation(
                                out=za[:, b * et:(b + 1) * et],
                                in_=gt[:, b * et:(b + 1) * et],
                                func=mybir.ActivationFunctionType.Abs)

                        # logits + exp, 512-edge chunks
                        for ch0 in range(0, et if edge_stage >= 3 else 0, 512):
                            cw = min(512, et - ch0)
                            lg = psp.tile([128, 512], fp32, tag="lg", name="lg", bufs=3)
                            nc.tensor.matmul(out=lg[:16, :cw], lhsT=a4z_s[:, 0, 0:16],
                                             rhs=gt[:, ch0:ch0 + cw],
                                             start=True, stop=False)
                            nc.tensor.matmul(out=lg[:16, :cw], lhsT=a4z_s[:, 1, 0:16],
                                             rhs=gt[:, et + ch0:et + ch0 + cw],
                                             start=False, stop=False)
                            nc.tensor.matmul(out=lg[:16, :cw], lhsT=a4a_s[:, 0, 0:16],
                                             rhs=za[:, ch0:ch0 + cw],
                                             start=False, stop=False)
                            nc.tensor.matmul(out=lg[:16, :cw], lhsT=a4a_s[:, 1, 0:16],
                                             rhs=za[:, et + ch0:et + ch0 + cw],
                                             start=False, stop=True)
                            nc.scalar.activation(
                                out=w16[:16, ch0:ch0 + cw], in_=lg[:16, :cw],
                                func=mybir.ActivationFunctionType.Exp)

                        if edge_stage >= 3:
                            wt = psp.tile([128, (SUBMAX // 128) * 16], bf16,
                                          tag="wt", name="wt")
                            for n in range(nch):
                                nc.tensor.transpose(
                                    out=wt[:, n * 16:(n + 1) * 16],
                                    in_=w16[0:16, n * 128:(n + 1) * 128],
                                    identity=identj_sb[0:16, :])
                            pend_wn.append((wn, wt, nch))
                            if len(pend_wn) > 1:
                                wn_, wt_, nch_ = pend_wn.pop(0)
                                nc.vector.tensor_copy(wn_[:], wt_[:, :nch_ * 16])
                        pend.append(dict(t=t, nch=nch, gn=gn, wn=wn, mk=mk,
                                         pw=pw, ci0=chunk_i,
                                         nct=n_chunks_tile,
                                         last=(si == n_sub - 1)))
                        chunk_i += nch
                        if len(pend) > PIPE_LAG:
                            flush_b()
                while pend_wn:
                    wn_, wt_, nch_ = pend_wn.pop(0)
                    nc.vector.tensor_copy(wn_[:], wt_[:, :nch_ * 16])
                while pend:
                    flush_b()

                # ---- GraphNorm stats (global) + h + skip
                s12 = sp.tile([128, 4], fp32, tag="s12", name="s12")  # [S1b0 S2b0 S1b1 S2b1]
                sq = sp.tile([128, NL], fp32, tag="sq", name="sq")
                for b in range(2):
                    nc.vector.tensor_reduce(
                        out=s12[:, 2 * b:2 * b + 1], in_=outT[b][:],
                        axis=mybir.AxisListType.X, op=mybir.AluOpType.add)
                    nc.vector.tensor_tensor(out=sq[:], in0=outT[b][:],
                                            in1=outT[b][:],
                                            op=mybir.AluOpType.mult)
                    nc.vector.tensor_reduce(
                        out=s12[:, 2 * b + 1:2 * b + 2], in_=sq[:],
                        axis=mybir.AxisListType.X, op=mybir.AluOpType.add)
                for b in range(2):
                    nc.sync.dma_start(out=st_in[b * 128:(b + 1) * 128, :],
                                      in_=s12[:, 2 * b:2 * b + 2])
                if do_coll:
                    nc.gpsimd.collective_compute(
                        "AllReduce", mybir.AluOpType.add,
                        replica_groups=groups, ins=[st_in[:]], outs=[st_out[:]])
                else:
                    nc.sync.dma_start(out=st_out[:], in_=st_in[:])
                s12g = sp.tile([128, 4], fp32, tag="s12g", name="s12g")
                for b in range(2):
                    nc.sync.dma_start(out=s12g[:, 2 * b:2 * b + 2],
                                      in_=st_out[b * 128:(b + 1) * 128, :])

                c1 = sp.tile([128, 2], fp32, tag="c1", name="c1")
                var = sp.tile([128, 2], fp32, tag="var", name="var")
                rstd = sp.tile([128, 2], fp32, tag="rstd", name="rstd")
                f_ = sp.tile([128, 2], fp32, tag="f_", name="f_")
                for b in range(2):
                    S1 = s12g[:, 2 * b:2 * b + 1]
                    S2 = s12g[:, 2 * b + 1:2 * b + 2]
                    # c1 = A + B*S1
                    nc.vector.tensor_tensor(out=c1[:, b:b + 1],
                                            in0=S1, in1=cbB_s[:, b:b + 1],
                                            op=mybir.AluOpType.mult)
                    nc.vector.tensor_tensor(out=c1[:, b:b + 1],
                                            in0=c1[:, b:b + 1],
                                            in1=cbA_s[:, b:b + 1],
                                            op=mybir.AluOpType.add)
                    # var = S2/N + c1*(2*S1/N + c1)
                    nc.vector.tensor_scalar(
                        out=var[:, b:b + 1], in0=S1, scalar1=2.0 / N,
                        scalar2=None, op0=mybir.AluOpType.mult)
                    nc.vector.tensor_tensor(out=var[:, b:b + 1],
                                            in0=var[:, b:b + 1],
                                            in1=c1[:, b:b + 1],
                                            op=mybir.AluOpType.add)
                    nc.vector.tensor_tensor(out=var[:, b:b + 1],
                                            in0=var[:, b:b + 1],
                                            in1=c1[:, b:b + 1],
                                            op=mybir.AluOpType.mult)
                    nc.vector.tensor_scalar(
                        out=var[:, b:b + 1], in0=S2, scalar1=1.0 / N,
                        scalar2=var[:, b:b + 1], op0=mybir.AluOpType.mult,
                        op1=mybir.AluOpType.add)
                    # rstd = 1/sqrt(var + eps)
                    nc.scalar.activation(
                        out=rstd[:, b:b + 1], in_=var[:, b:b + 1],
                        func=mybir.ActivationFunctionType.Sqrt, bias=eps_t[:])
                    nc.vector.reciprocal(rstd[:, b:b + 1], rstd[:, b:b + 1])
                    nc.vector.tensor_tensor(out=f_[:, b:b + 1],
                                            in0=rstd[:, b:b + 1],
                                            in1=gnw_s[:, b:b + 1],
                                            op=mybir.AluOpType.mult)
                    # h = (out + c1) * f + gnb  (into hT)
                    nc.vector.tensor_scalar(
                        out=hT[b][:], in0=outT[b][:],
                        scalar1=c1[:, b:b + 1], scalar2=None,
                        op0=mybir.AluOpType.add)
                    nc.vector.tensor_scalar(
                        out=hT[b][:], in0=hT[b][:],
                        scalar1=f_[:, b:b + 1], scalar2=gnb_s[:, b:b + 1],
                        op0=mybir.AluOpType.mult, op1=mybir.AluOpType.add)
                    # elu(h) - 1 fold: h' = relu(h) + exp(min(h,0))
                    nc.vector.tensor_scalar(
                        out=sq[:], in0=hT[b][:], scalar1=0.0, scalar2=None,
                        op0=mybir.AluOpType.min)
                    nc.scalar.activation(
                        out=sq[:], in_=sq[:],
                        func=mybir.ActivationFunctionType.Exp)
                    nc.vector.tensor_scalar(
                        out=hT[b][:], in0=hT[b][:], scalar1=0.0, scalar2=None,
                        op0=mybir.AluOpType.max)
                    nc.vector.tensor_tensor(
                        out=hT[b][:], in0=hT[b][:], in1=sq[:],
                        op=mybir.AluOpType.add)

                # skip matmul: xT += W_sk.T h' + bsk_eff
                for bo in range(2):
                    for ch0 in range(0, NL, 512):
                        cw = min(512, NL - ch0)
                        ps = psp.tile([128, 512], fp32, tag="wsum", name="wsum")
                        for bi in range(2):
                            nc.tensor.matmul(
                                out=ps[:, :cw],
                                lhsT=wsk_s[:, bi, bo],
                                rhs=hT[bi][:, ch0:ch0 + cw],
                                start=(bi == 0), stop=(bi == 1))
                        nc.vector.tensor_tensor(
                            out=xT[bo][:, ch0:ch0 + cw],
                            in0=xT[bo][:, ch0:ch0 + cw], in1=ps[:, :cw],
                            op=mybir.AluOpType.add)
                    nc.vector.tensor_scalar(
                        out=xT[bo][:], in0=xT[bo][:],
                        scalar1=bsk_s[:, bo:bo + 1], scalar2=None,
                        op0=mybir.AluOpType.add)
                    nc.vector.tensor_copy(xTb[bo][:], xT[bo][:])

            fcw_s = pp.tile([128, 2], bf16, tag="fcws", name="fcws")
            fcb_s = pp.tile([1, 1], fp32, tag="fcbs", name="fcbs")
            nc.sync.dma_start(out=fcw_s[:], in_=v_pb2("fcw_t"))
            nc.sync.dma_start(out=fcb_s[:], in_=fcb_ap)

            for l in range(n_layers):
                layer(l)

            # final fc
            y_sb = sp.tile([1, NL], fp32, tag="ysb", name="ysb")
            for ch0 in range(0, NL, 512):
                cw = min(512, NL - ch0)
                ps = psp.tile([128, 512], fp32, tag="lg", name="lg", bufs=3)
                for b in range(2):
                    nc.tensor.matmul(out=ps[:1, :cw], lhsT=fcw_s[:, b:b + 1],
                                     rhs=xTb[b][:, ch0:ch0 + cw],
                                     start=(b == 0), stop=(b == 1))
                nc.scalar.activation(
                    out=y_sb[:, ch0:ch0 + cw], in_=ps[:1, :cw],
                    func=mybir.ActivationFunctionType.Identity,
                    bias=fcb_s[:], scale=1.0)
            nc.sync.dma_start(out=y_out[:], in_=y_sb[:])

    nc.compile()
    return nc


# ---------------------------------------------------------------- runner

_CACHE = {}


def kernel(**inputs):
    layout, in_maps = prep_inputs(inputs)
    key = (layout["NL"], layout["E_pad"],
           tuple(int(x) for x in layout["tile_edge_base"]))
    if key not in _CACHE:
        _CACHE[key] = build_program(layout)
    nc = _CACHE[key]
    res = bass_utils.run_bass_kernel_spmd(nc, in_maps, core_ids=list(range(NCORES)))
    y = np.zeros(N, np.float32)
    for c in range(NCORES):
        yc = np.asarray(res.results[c]["y"], np.float32).reshape(-1)
        y[layout["core_nodes"][c]] = yc[layout["cores"][c]["slot_of_node"]]
    return y


if __name__ == "__main__":
    sys.path.insert(0, "/root/problem")
    import jax
    import reference

    with jax.default_device(jax.devices("cpu")[0]):
        inputs = {k: np.asarray(v) for k, v in reference.setup_inputs().items()}
        expected = np.asarray(reference.reference(**inputs))
    got = kernel(**inputs)
    rel = np.linalg.norm(got - expected) / np.linalg.norm(expected)
    print("rel l2:", rel)
    print(expected[:4], got[:4])



# revision 59
# speedup vs baseline: 1.0766x; 1.0620x over previous
"""Trainium2 Bass kernel for nn_EnhancedFinGAT (4-layer GATv2 + GraphNorm + skip).

Strategy (8 NeuronCores, SPMD):
  - Nodes (and their incoming edges) are sharded by destination across the 8
    cores; per-core nodes are permuted into degree-bucket-major "slots" so the
    per-edge xr[dst] add becomes a broadcast access pattern (no second gather).
  - Per layer: local matmuls produce xl (normal layout, AllGathered into a
    DRAM table) and xr (transposed, SBUF).  Edge phase per 128-dst tile:
    one transposed dma_gather (for PE logits) + one normal dma_gather (for the
    PE mask-matmul weighted segment sum), leaky-relu via the |z| decomposition,
    softmax without max-subtraction (logits are bounded), exp on ScalarE,
    per-128-edge mask matmuls accumulate both the weighted sums and the
    softmax denominators in PSUM.
  - GraphNorm stats via one AllReduce of (sum, sum-of-squares); skip matmul in
    transposed layout.
All heavy per-edge data is bf16; accumulations are f32.
"""

import os
import sys
import numpy as np

sys.path.insert(0, "/opt/trn_rl_repo")

import concourse.bass as bass
import concourse.bacc as bacc
import concourse.mybir as mybir
import concourse.tile as tile
import concourse.bass_utils as bass_utils
from concourse.masks import make_identity

fp32 = mybir.dt.float32
f32r = mybir.dt.float32r
bf16 = mybir.dt.bfloat16
i16 = mybir.dt.int16

N, HID, L, H, CH = 10000, 256, 4, 4, 64
NCORES = 8
NPC = N // NCORES
EPS = 1e-5
P = 128


# ---------------------------------------------------------------- host prep

def _bucket_of(deg):
    if deg <= 128:
        return max(2, ((deg + 1) // 2) * 2)
    for k in (136, 144, 152, 160, 176, 192, 208, 224, 240, 256, 320, 384, 512):
        if deg <= k:
            return k
    raise ValueError(deg)


def _cumcount(x):
    n = len(x)
    if n == 0:
        return np.zeros(0, np.int64)
    change = np.empty(n, dtype=bool)
    change[0] = True
    change[1:] = x[1:] != x[:-1]
    run_starts = np.flatnonzero(change)
    return np.arange(n) - run_starts[np.cumsum(change) - 1]


def build_layout(edge_index, sub_cap=2048):
    src_g = np.concatenate([np.asarray(edge_index[0], np.int64), np.arange(N)])
    dst_g = np.concatenate([np.asarray(edge_index[1], np.int64), np.arange(N)])

    # degree-balanced node->core assignment: deal nodes (sorted by degree)
    # round-robin so the per-core bucket histograms almost coincide.
    deg_g = np.bincount(dst_g, minlength=N)
    order = np.argsort(-deg_g, kind="stable")
    core_nodes = [np.sort(order[c::NCORES]) for c in range(NCORES)]
    loc_of = np.zeros(N, np.int64)
    core_of_node = np.zeros(N, np.int64)
    for c in range(NCORES):
        loc_of[core_nodes[c]] = np.arange(NPC)
        core_of_node[core_nodes[c]] = c
    core_of = core_of_node[dst_g]

    per_core = []
    all_buckets = {}
    for c in range(NCORES):
        m = core_of == c
        s, d = src_g[m], loc_of[dst_g[m]]
        deg = np.bincount(d, minlength=NPC)
        buckets = np.array([_bucket_of(x) for x in deg])
        cnt = {}
        for k in buckets:
            cnt[int(k)] = cnt.get(int(k), 0) + 1
        for k, v in cnt.items():
            all_buckets[k] = max(all_buckets.get(k, 0), v)
        per_core.append((s, d, buckets))

    ks = sorted(all_buckets)
    M = {k: all_buckets[k] for k in ks}
    NL = ((sum(M.values()) + P - 1) // P) * P
    NT = NCORES * NL
    PAD_ROW = 0

    slot_bucket = np.zeros(NL, np.int64)
    off = 0
    bucket_slot_base = {}
    for k in ks:
        bucket_slot_base[k] = off
        slot_bucket[off:off + M[k]] = k
        off += M[k]

    n_tiles = NL // P
    SUB_CAP = sub_cap
    slot_edge_off = np.zeros(NL, np.int64)
    tile_edge_base = np.zeros(n_tiles + 1, np.int64)
    tile_subs = []  # per tile: list of (e_start, e_end, [(k, d0, m), ...])
    e = 0
    for t in range(n_tiles):
        tile_edge_base[t] = e
        subs = []
        sub_start = e
        sub_runs = []
        run = None  # (k, d0, m)
        for d in range(t * P, (t + 1) * P):
            k = int(slot_bucket[d])
            if k == 0:
                continue
            pad_now = ((e + P - 1) // P) * P
            if pad_now + k - sub_start > SUB_CAP:
                # close current sub before this node
                if run is not None:
                    sub_runs.append(run)
                    run = None
                e = pad_now
                subs.append((int(sub_start), int(e), sub_runs))
                sub_runs = []
                sub_start = e
            slot_edge_off[d] = e
            if run is not None and run[0] == k:
                run = (k, run[1], run[2] + 1)
            else:
                if run is not None:
                    sub_runs.append(run)
                run = (k, d, 1)
            e += k
        if run is not None:
            sub_runs.append(run)
        if e > sub_start or sub_runs:
            e = ((e + P - 1) // P) * P
            subs.append((int(sub_start), int(e), sub_runs))
        tile_subs.append(subs)
    tile_edge_base[n_tiles] = e
    E_pad = int(e)

    cores = []
    for c in range(NCORES):
        s, d, buckets = per_core[c]
        slot_of_node = np.full(NPC, -1, np.int64)
        next_free = dict(bucket_slot_base)
        for n_loc in np.argsort(buckets, kind="stable"):
            k = int(buckets[n_loc])
            slot_of_node[n_loc] = next_free[k]
            next_free[k] += 1
        order = np.argsort(slot_of_node[d], kind="stable")
        cores.append(dict(slot_of_node=slot_of_node,
                          s_sorted=s[order],
                          d_sorted_slot=slot_of_node[d][order]))

    g2p = np.zeros(N, np.int64)
    for c in range(NCORES):
        g2p[core_nodes[c]] = c * NL + cores[c]["slot_of_node"]

    # run membership arrays: edges of a run (k, d0, m) are interleaved
    # dst-major (edge j of dst i at run_base + j*m + i) so the xr[dst]
    # broadcast add has a packed last dim (DVE 2x mode).
    run_d0 = np.arange(NL)
    run_m = np.ones(NL, np.int64)
    for subs in tile_subs:
        for (_e0, _e1, runs) in subs:
            for (k, d0, m) in runs:
                run_d0[d0:d0 + m] = d0
                run_m[d0:d0 + m] = m

    for c in range(NCORES):
        src_slot = np.full(E_pad, PAD_ROW, np.int64)
        dst_slot = np.full(E_pad, -1, np.int64)
        d_sl = cores[c]["d_sorted_slot"]
        pos = (slot_edge_off[run_d0[d_sl]] + _cumcount(d_sl) * run_m[d_sl]
               + (d_sl - run_d0[d_sl]))
        src_slot[pos] = g2p[cores[c]["s_sorted"]]
        dst_slot[pos] = d_sl
        cores[c]["src_slot"] = src_slot
        cores[c]["dst_slot_of_edge"] = dst_slot

    return dict(NL=int(NL), NT=int(NT), PAD_ROW=int(PAD_ROW), E_pad=E_pad,
                n_tiles=n_tiles, tile_edge_base=tile_edge_base,
                tile_subs=tile_subs, slot_edge_off=slot_edge_off,
                core_nodes=core_nodes, g2p=g2p, sub_cap=int(SUB_CAP)), cores


def wrap_idx16(idx):
    n = len(idx)
    cols = (n + 15) // 16
    pad = np.zeros(cols * 16, np.int64)
    pad[:n] = idx
    w = pad.reshape(cols, 16).T.astype(np.int16)
    return np.tile(w, (8, 1))


def build_masks(layout, core):
    E_pad = layout["E_pad"]
    n_chunks = E_pad // P
    dst = core["dst_slot_of_edge"]
    masks = np.zeros((n_chunks, P, P), np.float32)
    for chn in range(n_chunks):
        d = dst[chn * P:(chn + 1) * P]
        rows = np.flatnonzero(d >= 0)
        masks[chn, rows, (d[d >= 0] % P)] = 1.0
    return masks


def att4_lhst(att_l, scale):
    # features are channel-interleaved: feature f holds (h, c) = (f%4, f//4)
    out = np.zeros((2, P, P), np.float32)
    for b in range(2):
        for p in range(P):
            f = 128 * b + p
            h = f % H
            out[b, p, h::4] = scale * att_l[h, f // H]
    return out


def _to_bf16(x):
    import jax.numpy as jnp
    return np.asarray(jnp.asarray(x, jnp.bfloat16)).view(np.uint16)


# numpy bf16 arrays are passed as uint16 views?  Simpler: use ml_dtypes.
def to_bf16(x):
    import ml_dtypes
    return np.asarray(x, np.float32).astype(ml_dtypes.bfloat16)


def prep_inputs(inputs, sub_cap=2048):
    """Returns (layout, in_maps) — one dict per core."""
    layout, cores = build_layout(np.asarray(inputs["edge_index"]), sub_cap=sub_cap)
    NL, E_pad = layout["NL"], layout["E_pad"]

    x = np.asarray(inputs["x"], np.float32)
    lw = np.asarray(inputs["lin_l_w"], np.float32)
    lb = np.asarray(inputs["lin_l_b"], np.float32)
    rw = np.asarray(inputs["lin_r_w"], np.float32)
    rb = np.asarray(inputs["lin_r_b"], np.float32)
    att = np.asarray(inputs["att"], np.float32)
    cb = np.asarray(inputs["conv_bias"], np.float32)
    gnw = np.asarray(inputs["gn_weight"], np.float32)
    gnb = np.asarray(inputs["gn_bias"], np.float32)
    gnm = np.asarray(inputs["gn_mean_scale"], np.float32)
    skw = np.asarray(inputs["skip_w"], np.float32)

    # channel-interleaved h-space: new feature c*H+h <- old h*CH+c
    PERM = np.zeros(HID, np.int64)
    for h in range(H):
        PERM[np.arange(CH) * H + h] = h * CH + np.arange(CH)
    lw = lw[:, :, PERM]
    lb = lb[:, PERM]
    rw = rw[:, :, PERM]
    rb = rb[:, PERM]
    cb = cb[:, PERM]
    gnw = gnw[:, PERM]
    gnb = gnb[:, PERM]
    gnm = gnm[:, PERM]
    skw = skw[PERM, :]
    skb = np.asarray(inputs["skip_b"], np.float32)
    fcw = np.asarray(inputs["fc_w"], np.float32)
    fcb = np.asarray(inputs["fc_b"], np.float32)

    # layer-stacked common weights (same on all cores)
    wl_in = lw.reshape(L, 2, 128, 256)                    # [l, bi, 128, 256]
    wr_in = rw.reshape(L, 2, 128, 256)
    wsk_in = skw.reshape(2, 128, 2, 128).transpose(0, 2, 1, 3)  # [bi, bo, 128, 128]
    bl_in = lb.reshape(L, 1, 256)
    br_t = rb.reshape(L, 2, 128, 1)
    bsk_eff = (skb - skw.sum(axis=0)).reshape(2, 128, 1)
    a4z = np.stack([att4_lhst(att[l], 0.6) for l in range(L)])  # [L, 2, 128, 128]
    a4a = np.stack([att4_lhst(att[l], 0.4) for l in range(L)])
    cbA = (cb * (1.0 - gnm)).reshape(L, 2, 128, 1)
    cbB = np.broadcast_to((-gnm / float(N)).reshape(L, 2, 128, 1), (L, 2, 128, 1))
    gnw_t = gnw.reshape(L, 2, 128, 1)
    gnb_t = gnb.reshape(L, 2, 128, 1)
    fcw_t = fcw.reshape(2, 128, 1)
    fcb_in = fcb.reshape(1, 1)

    identj = np.zeros((128, 16), np.float32)
    for j in range(3):
        identj[32 * j:32 * j + 16] = np.eye(16)

    common = dict(
        wl=to_bf16(wl_in), wr=to_bf16(wr_in), wsk=to_bf16(wsk_in.copy()),
        bl=to_bf16(bl_in), br_t=br_t,
        bsk=bsk_eff, a4z=to_bf16(a4z), a4a=to_bf16(a4a),
        cbA=cbA, cbB=np.asarray(cbB, np.float32).copy(), gnw_t=gnw_t, gnb_t=gnb_t,
        fcw_t=to_bf16(fcw_t), fcb=fcb_in, identj=to_bf16(identj),
    )

    in_maps = []
    for c in range(NCORES):
        core = cores[c]
        x0 = np.zeros((NL, HID), np.float32)
        x0[core["slot_of_node"]] = x[layout["core_nodes"][c]]
        x0t = x0.T.reshape(2, 128, NL).copy()
        m = dict(common)
        m["x0t"] = x0t
        m["iotaw"] = wrap_idx16(np.arange(layout["sub_cap"]))
        m["srcw"] = wrap_idx16(core["src_slot"])
        d2 = core["dst_slot_of_edge"].reshape(-1, 128).T
        m["dsts"] = np.where(d2 >= 0, d2 % 128, -1).astype(np.float32)
        in_maps.append(pack_core(layout, m))

    layout["cores"] = cores
    return layout, in_maps


def pack_spec(layout):
    """Fixed packing of all external inputs into 3 dtype buffers.
    Returns name -> (buf_key, offset, shape); buf keys: pb (bf16), pf (f32),
    pi (i16)."""
    NL, E_pad = layout["NL"], layout["E_pad"]
    EW = (E_pad + 15) // 16
    NCH = E_pad // P
    sc = layout["sub_cap"]
    specs = {}
    offs = {"pb": 0, "pf": 0, "pi": 0}

    def add(key, name, shape):
        specs[name] = (key, offs[key], tuple(shape))
        offs[key] += int(np.prod(shape))

    add("pb", "wl", (L, 2, 128, 256))
    add("pb", "wr", (L, 2, 128, 256))
    add("pb", "wsk", (2, 2, 128, 128))
    add("pb", "bl", (L, 1, 256))
    add("pb", "a4z", (L, 2, 128, 128))
    add("pb", "a4a", (L, 2, 128, 128))
    add("pb", "fcw_t", (2, 128, 1))
    add("pb", "identj", (128, 16))
    add("pf", "x0t", (2, 128, NL))
    add("pf", "br_t", (L, 2, 128, 1))
    add("pf", "bsk", (2, 128, 1))
    add("pf", "cbA", (L, 2, 128, 1))
    add("pf", "cbB", (L, 2, 128, 1))
    add("pf", "gnw_t", (L, 2, 128, 1))
    add("pf", "gnb_t", (L, 2, 128, 1))
    add("pf", "fcb", (1, 1))
    add("pf", "dsts", (128, NCH))
    add("pi", "srcw", (128, EW))
    add("pi", "iotaw", (128, sc // 16))
    return specs, dict(offs)


def pack_core(layout, tensors):
    import ml_dtypes
    specs, sizes = pack_spec(layout)
    out = {
        "pb": np.zeros((1, sizes["pb"]), ml_dtypes.bfloat16),
        "pf": np.zeros((1, sizes["pf"]), np.float32),
        "pi": np.zeros((1, sizes["pi"]), np.int16),
    }
    for name, (key, off, shape) in specs.items():
        arr = tensors[name]
        assert tuple(arr.shape) == tuple(shape), (name, arr.shape, shape)
        out[key][0, off:off + arr.size] = np.asarray(arr).reshape(-1)
    return out


# ---------------------------------------------------------------- device build

def build_program(layout, n_layers=L, do_edges=True, do_coll=True, edge_stage=5,
                  do_gt=True, do_gn=True, do_mk=True, spk=False, gt_from_sbuf=False,
                  gt_from_table=False, alt_queues=False, split_q=False, gn_from_gt=False,
                  dma_scratch=16384):
    NL, NT, E_pad = layout["NL"], layout["NT"], layout["E_pad"]
    n_tiles = layout["n_tiles"]
    teb = layout["tile_edge_base"]
    EW = (E_pad + 15) // 16
    NCH = E_pad // P
    SUBMAX = 128 * max(
        (s[1] - s[0]) // 128 for subs in layout["tile_subs"] for s in subs)

    nc = bacc.Bacc("TRN2", target_bir_lowering=False, debug=False,
                   num_devices=NCORES, num_swdge_queues=4 if (alt_queues or split_q) else 2,
                   dynamic_dma_scratch_size=dma_scratch)

    # ---- I/O: all external inputs packed into 3 dtype buffers
    specs, sizes = pack_spec(layout)
    pb = nc.dram_tensor("pb", [1, sizes["pb"]], bf16, kind="ExternalInput")
    pf = nc.dram_tensor("pf", [1, sizes["pf"]], fp32, kind="ExternalInput")
    pi = nc.dram_tensor("pi", [1, sizes["pi"]], i16, kind="ExternalInput")
    bufs = {"pb": pb, "pf": pf, "pi": pi}

    def pslice(name, idx=None):
        """Flat [1, numel] AP of tensor `name`, optionally of slice [idx]
        along dim 0."""
        key, off, shape = specs[name]
        numel = int(np.prod(shape))
        if idx is None:
            return bufs[key][0:1, off:off + numel]
        sub = numel // shape[0]
        return bufs[key][0:1, off + idx * sub:off + (idx + 1) * sub]

    def v_pbc(name, idx, b, p_, c):      # [l] . "b p c -> p b c"
        return pslice(name, idx).rearrange("o (b p c) -> (o p) b c", b=b, p=p_, c=c)

    def v_pb2(name, idx=None):           # [l] . "b p o -> p (b o)" (o==1)
        return pslice(name, idx).rearrange("z (b p) -> (z p) b", b=2, p=128)

    x0t_ap = lambda b: pslice("x0t", b).rearrange("o (p n) -> (o p) n", p=128)
    srcw_ap = pslice("srcw").rearrange("o (p e) -> (o p) e", p=128)
    iotaw_ap = pslice("iotaw").rearrange("o (p e) -> (o p) e", p=128)
    dsts_ap = pslice("dsts").rearrange("o (p n) -> (o p) n", p=128, n=NCH)
    identj_ap = pslice("identj").rearrange("o (p c) -> (o p) c", p=128)
    fcb_ap = pslice("fcb")

    y_out = nc.dram_tensor("y", [1, NL], fp32, kind="ExternalOutput")

    # ---- internal DRAM
    xl_bounce = nc.dram_tensor("xl_bounce", [NL, 256], bf16, kind="Internal")
    masks_dram = nc.dram_tensor("masks_dram", [128, NCH, 128], bf16, kind="Internal")
    st_in = nc.dram_tensor("st_in", [256, 2], fp32, kind="Internal")
    st_out = nc.dram_tensor("st_out", [256, 2], fp32, kind="Internal",
                            addr_space="Shared")
    xlt_sh = nc.dram_tensor("xlt_sh", [NT, 256], bf16, kind="Internal",
                            addr_space="Shared")

    groups = [list(range(NCORES))]

    with tile.TileContext(nc) as tc:
        with tc.tile_pool(name="persist", bufs=1) as pp, \
             tc.tile_pool(name="work", bufs=2) as wp, \
             tc.tile_pool(name="single", bufs=1) as sp, \
             tc.tile_pool(name="psum", bufs=2, space="PSUM") as psp:

            # ---------------- constants / persistent state
            ident128 = pp.tile([128, 128], fp32, tag="id128", name="id128")
            make_identity(nc, ident128[:])
            ident128b = pp.tile([128, 128], bf16, tag="id128b", name="id128b")
            nc.vector.tensor_copy(ident128b[:], ident128[:])
            identj_sb = pp.tile([128, 16], bf16, tag="idj", name="idj")
            nc.sync.dma_start(out=identj_sb[:], in_=identj_ap)

            ones_row = pp.tile([1, 128], bf16, tag="ones", name="ones")
            eps_t = pp.tile([128, 1], fp32, tag="epsT", name="epsT")
            nc.gpsimd.memset(eps_t[:], EPS)
            nc.gpsimd.memset(ones_row[:], 1.0)

            xT = [pp.tile([128, NL], fp32, tag=f"xT{b}", name=f"xT{b}") for b in range(2)]
            xrT = [pp.tile([128, NL], bf16, tag=f"xrT{b}", name=f"xrT{b}") for b in range(2)]
            xrTf = [pp.tile([128, NL], fp32, tag=f"xrTf{b}", name=f"xrTf{b}") for b in range(2)]
            xTb = [pp.tile([128, NL], bf16, tag=f"xTb{b}", name=f"xTb{b}") for b in range(2)]
            outT = [pp.tile([128, NL], fp32, tag=f"outT{b}", name=f"outT{b}") for b in range(2)]
            hT = [pp.tile([128, NL], bf16, tag=f"hT{b}", name=f"hT{b}") for b in range(2)]
            xl_sb = pp.tile([128, (NL // 128) * 256], bf16, tag="xlsb", name="xlsb")
            xlt_sb = (pp.tile([128, NT // 128, 256], bf16, tag="xltsb", name="xltsb")
                      if gt_from_table else None)
            srcw_sb = pp.tile([128, EW], i16, tag="srcsb", name="srcsb")
            nc.sync.dma_start(out=srcw_sb[:], in_=srcw_ap)
            iotaw_sb = pp.tile([128, layout["sub_cap"] // 16], i16, tag="iotasb", name="iotasb")
            nc.sync.dma_start(out=iotaw_sb[:], in_=iotaw_ap)
            dsts_sb = pp.tile([128, NCH], fp32, tag="dstssb", name="dstssb")
            nc.sync.dma_start(out=dsts_sb[:], in_=dsts_ap)
            iota_d = pp.tile([128, 128], fp32, tag="iotad", name="iotad")
            nc.gpsimd.iota(iota_d[:], pattern=[[1, 128]], base=0,
                           channel_multiplier=0,
                           allow_small_or_imprecise_dtypes=True)

            for b in range(2):
                nc.sync.dma_start(out=xT[b][:], in_=x0t_ap(b))
                nc.vector.tensor_copy(xTb[b][:], xT[b][:])

            # per-layer weight staging
            wl_s = pp.tile([128, 2, 256], bf16, tag="wls", name="wls")
            wr_s = pp.tile([128, 2, 256], bf16, tag="wrs", name="wrs")
            wsk_s = pp.tile([128, 2, 2, 128], bf16, tag="wsks", name="wsks")
            bl_s = pp.tile([1, 256], bf16, tag="bls", name="bls")
            br_s = pp.tile([128, 2], fp32, tag="brs", name="brs")
            bsk_s = pp.tile([128, 2], fp32, tag="bsks", name="bsks")
            a4z_s = pp.tile([128, 2, 128], bf16, tag="a4zs", name="a4zs")
            a4a_s = pp.tile([128, 2, 128], bf16, tag="a4as", name="a4as")
            cbA_s = pp.tile([128, 2], fp32, tag="cbAs", name="cbAs")
            cbB_s = pp.tile([128, 2], fp32, tag="cbBs", name="cbBs")
            gnw_s = pp.tile([128, 2], fp32, tag="gnws", name="gnws")
            gnb_s = pp.tile([128, 2], fp32, tag="gnbs", name="gnbs")

            for b in range(2):
                nc.sync.dma_start(out=wsk_s[:, b], in_=pslice("wsk", b).rearrange("z (o p c) -> (z p) o c", o=2, p=128, c=128))
            nc.sync.dma_start(out=bsk_s[:], in_=v_pb2("bsk"))

            def layer(l):
                # ---- stage layer weights
                nc.sync.dma_start(out=wl_s[:], in_=v_pbc("wl", l, 2, 128, 256))
                nc.sync.dma_start(out=wr_s[:], in_=v_pbc("wr", l, 2, 128, 256))
                nc.sync.dma_start(out=bl_s[:], in_=pslice("bl", l))
                nc.sync.dma_start(out=br_s[:], in_=v_pb2("br_t", l))
                nc.sync.dma_start(out=a4z_s[:], in_=v_pbc("a4z", l, 2, 128, 128))
                nc.sync.dma_start(out=a4a_s[:], in_=v_pbc("a4a", l, 2, 128, 128))
                nc.sync.dma_start(out=cbA_s[:], in_=v_pb2("cbA", l))
                nc.sync.dma_start(out=cbB_s[:], in_=v_pb2("cbB", l))
                nc.sync.dma_start(out=gnw_s[:], in_=v_pb2("gnw_t", l))
                nc.sync.dma_start(out=gnb_s[:], in_=v_pb2("gnb_t", l))

                # ---- xl (normal layout) and xr (transposed) from x_T
                for t in range(NL // 128):
                    ps = psp.tile([128, 512], fp32, tag="lg", name="lg", bufs=3)
                    for bi in range(2):
                        nc.tensor.matmul(out=ps[:, :256],
                                         lhsT=xTb[bi][:, t * 128:(t + 1) * 128],
                                         rhs=wl_s[:, bi],
                                         start=(bi == 0), stop=False)
                    nc.tensor.matmul(out=ps[:, :256], lhsT=ones_row[:],
                                     rhs=bl_s[:], start=False, stop=True)
                    nc.vector.tensor_copy(
                        xl_sb[:, t * 256:(t + 1) * 256], ps[:, :256])
                nc.sync.dma_start(
                    out=xl_bounce[:].rearrange("(t p) c -> p t c", p=128),
                    in_=xl_sb[:].rearrange("p (t c) -> p t c", c=256))

                # xr transposed: out block bo over node chunks of 512
                for bo in range(2):
                    for ch0 in range(0, NL, 512):
                        cw = min(512, NL - ch0)
                        ps = psp.tile([128, 512], fp32, tag="wsum", name="wsum")
                        for bi in range(2):
                            nc.tensor.matmul(
                                out=ps[:, :cw],
                                lhsT=wr_s[:, bi, bo * 128:(bo + 1) * 128],
                                rhs=xTb[bi][:, ch0:ch0 + cw],
                                start=(bi == 0), stop=(bi == 1))
                        nc.scalar.activation(
                            out=xrT[bo][:, ch0:ch0 + cw], in_=ps[:, :cw],
                            func=mybir.ActivationFunctionType.Identity,
                            bias=br_s[:, bo:bo + 1], scale=1.0)
                        nc.scalar.activation(
                            out=xrTf[bo][:, ch0:ch0 + cw], in_=ps[:, :cw],
                            func=mybir.ActivationFunctionType.Identity,
                            bias=br_s[:, bo:bo + 1], scale=1.0)

                # ---- AllGather xl into the table
                if do_coll:
                    nc.gpsimd.collective_compute(
                        "AllGather", mybir.AluOpType.bypass,
                        replica_groups=groups,
                        ins=[xl_bounce[:]],
                        outs=[xlt_sh[:]],
                    )
                if gt_from_table:
                    nc.sync.dma_start(
                        out=xlt_sb[:],
                        in_=xlt_sh[:].rearrange("(t p) c -> p t c", p=128))

                # ---- edge phase, per dst-tile, sub-chunked
                sub_seq = [0]
                cpi = [0]
                if not do_edges or edge_stage < 5:
                    for b in range(2):
                        nc.gpsimd.memset(outT[b][:], 0.0)
                # software-pipelined emission: defer each sub's B-phase
                # (wg + mask matmuls + tile-post) by PIPE_LAG subs so in-order
                # engines don't stall on the long gt->wn chain.
                PIPE_LAG = 3
                pend = []
                pend_wn = []

                def flush_b():
                    d = pend.pop(0)
                    nch_ = d["nch"]
                    pw_ = d["pw"]
                    if edge_stage >= 4:
                        wg = wp.tile([128, nch_, 260], bf16, tag="wg", name="wg",
                                     padded_shape=[128, SUBMAX // 128, 260],
                                     bufs=3)
                        nc.vector.tensor_tensor(
                            out=wg[:, :, 0:256].rearrange("p n (c f) -> p n c f", f=4),
                            in0=d["gn"][:].rearrange("p n (c f) -> p n c f", f=4),
                            in1=d["wn"][:, :, 0:4].to_broadcast([128, nch_, 4, 64])
                                .rearrange("p n f c -> p n c f"),
                            op=mybir.AluOpType.mult)
                        nc.vector.tensor_copy(wg[:, :, 256:260], d["wn"][:, :, 0:4])
                        for n in range(nch_ if edge_stage >= 5 else 0):
                            nc.tensor.matmul(out=pw_[:, :260], lhsT=d["mk"][:, n],
                                             rhs=wg[:, n],
                                             start=(d["ci0"] + n == 0),
                                             stop=(d["ci0"] + n == d["nct"] - 1))
                    if d["last"] and edge_stage >= 5:
                        t_ = d["t"]
                        srec = wp.tile([128, 4], fp32, tag="srec", name="srec")
                        nc.vector.tensor_scalar(
                            out=srec[:], in0=pw_[:, 256:260], scalar1=1e-20,
                            scalar2=None, op0=mybir.AluOpType.add)
                        nc.vector.reciprocal(srec[:], srec[:])
                        outn = wp.tile([128, 256], fp32, tag="outn", name="outn")
                        nc.vector.tensor_tensor(
                            out=outn[:].rearrange("p (c f) -> p c f", f=4),
                            in0=pw_[:, 0:256].rearrange("p (c f) -> p c f", f=4),
                            in1=srec[:].to_broadcast([128, 4, 64])
                                .rearrange("p f c -> p c f"),
                            op=mybir.AluOpType.mult)
                        for b in range(2):
                            tp = psp.tile([128, 128], fp32, tag="tpo", name="tpo", bufs=1)
                            nc.tensor.transpose(
                                out=tp[:], in_=outn[:, b * 128:(b + 1) * 128],
                                identity=ident128[:])
                            nc.vector.tensor_copy(
                                outT[b][:, t_ * 128:(t_ + 1) * 128], tp[:])

                for t in range(n_tiles if do_edges else 0):
                    subs = layout["tile_subs"][t]
                    n_sub = len(subs)
                    chunk_i = 0
                    n_chunks_tile = sum((s[1] - s[0]) // 128 for s in subs)
                    for si, (e0, e1, runs) in enumerate(subs):
                        et = e1 - e0
                        nch = et // 128
                        if si == 0:
                            pw = psp.tile([128, 512], fp32, tag="wsum", name="wsum")
                        gbufs = {512: 6, 1024: 4, 2048: 3}.get(layout["sub_cap"], 2)
                        gt = wp.tile([128, 2 * et], bf16, tag="gt", name="gt",
                                     padded_shape=[128, 2 * SUBMAX], bufs=gbufs)
                        gn = wp.tile([128, nch, 256], bf16, tag="gn", name="gn",
                                     padded_shape=[128, SUBMAX // 128, 256],
                                     bufs=gbufs + 1)
                        mk = wp.tile([128, nch, 128], bf16, tag="mk", name="mk",
                                     padded_shape=[128, SUBMAX // 128, 128],
                                     bufs=gbufs + 1)
                        za = wp.tile([128, 2 * et], bf16, tag="za", name="za",
                                     padded_shape=[128, 2 * SUBMAX], bufs=2)
                        w16 = wp.tile([16, et], bf16, tag="w16", name="w16",
                                      padded_shape=[16, SUBMAX], bufs=gbufs + 1)
                        wn = wp.tile([128, nch, 16], bf16, tag="wn", name="wn",
                                     padded_shape=[128, SUBMAX // 128, 16],
                                     bufs=gbufs + 1)

                        qbase = (sub_seq[0] % 2) * 2 if alt_queues else 0
                        sub_seq[0] += 1
                        if do_gt and not gt_from_sbuf and not gt_from_table:
                            if split_q:
                                for b in range(2):
                                    nc.gpsimd.dma_gather(
                                        out_ap=gt[:, b * et:(b + 1) * et].rearrange("p (u e) -> p u e", u=1),
                                        in_ap=xlt_sh[:, b * 128:(b + 1) * 128],
                                        idxs_ap=srcw_sb[:, e0 // 16:e1 // 16],
                                        num_idxs=et, num_idxs_reg=et,
                                        elem_size=128, elem_step=256,
                                        transpose=True, single_packet=spk,
                                        queue_num=2 * b)
                            else:
                                nc.gpsimd.dma_gather(
                                    out_ap=gt[:].rearrange("p (b e) -> p b e", b=2), in_ap=xlt_sh[:],
                                    idxs_ap=srcw_sb[:, e0 // 16:e1 // 16],
                                    num_idxs=et, num_idxs_reg=et, elem_size=256,
                                    transpose=True, single_packet=spk,
                                    queue_num=qbase)
                        if do_gt and gt_from_table:
                            nc.gpsimd.dma_gather(
                                out_ap=gt[:].rearrange("p (b e) -> p b e", b=2),
                                in_ap=xlt_sb[:],
                                idxs_ap=srcw_sb[:, e0 // 16:e1 // 16],
                                num_idxs=et, num_idxs_reg=et, elem_size=256,
                                transpose=True, single_packet=spk,
                                sbuf_tokens_per_rank=128,
                                sbuf_free_dim_per_rank=512,
                                sbuf_free_dim_pad_per_rank=0,
                                sbuf_byte_offset=0)
                        if do_gt and gt_from_sbuf:
                            nc.gpsimd.dma_gather(
                                out_ap=gt[:].rearrange("p (b e) -> p b e", b=2),
                                in_ap=gn[:],
                                idxs_ap=iotaw_sb[:, :et // 16],
                                num_idxs=et, num_idxs_reg=et, elem_size=256,
                                transpose=True, single_packet=spk,
                                sbuf_tokens_per_rank=128,
                                sbuf_free_dim_per_rank=512,
                                sbuf_free_dim_pad_per_rank=0,
                                sbuf_byte_offset=0)
                        if do_gn and gn_from_gt:
                            for g4 in range(0, nch, 4):
                                gw4 = min(4, nch - g4)
                                for b in range(2):
                                    tpg = psp.tile([128, 512], bf16, tag="tpg",
                                                   name="tpg")
                                    for k in range(gw4):
                                        n = g4 + k
                                        nc.tensor.transpose(
                                            out=tpg[:, k * 128:(k + 1) * 128],
                                            in_=gt[:, b * et + n * 128:
                                                   b * et + (n + 1) * 128],
                                            identity=ident128b[:])
                                    dst_ap = gn[:, g4:g4 + gw4,
                                                b * 128:(b + 1) * 128]
                                    src_ap = tpg[:, :gw4 * 128].rearrange(
                                        "p (k c) -> p k c", k=gw4)
                                    if cpi[0] % 2 == 0:
                                        nc.vector.tensor_copy(dst_ap, src_ap)
                                    else:
                                        nc.scalar.activation(
                                            out=dst_ap, in_=src_ap,
                                            func=mybir.ActivationFunctionType.Copy)
                                    cpi[0] += 1
                        if do_gn and not gn_from_gt:
                            if split_q and et >= 256:
                                eh0 = (nch // 2) * 128
                                for (s0, s1, q) in ((0, eh0, 1), (eh0, et, 3)):
                                    nc.gpsimd.dma_gather(
                                        out_ap=gn[:, s0 // 128:s1 // 128],
                                        in_ap=xlt_sh[:],
                                        idxs_ap=srcw_sb[:, (e0 + s0) // 16:(e0 + s1) // 16],
                                        num_idxs=s1 - s0, num_idxs_reg=s1 - s0,
                                        elem_size=256,
                                        transpose=False, single_packet=spk,
                                        queue_num=q)
                            else:
                                nc.gpsimd.dma_gather(
                                    out_ap=gn[:], in_ap=xlt_sh[:],
                                    idxs_ap=srcw_sb[:, e0 // 16:e1 // 16],
                                    num_idxs=et, num_idxs_reg=et, elem_size=256,
                                    transpose=False, single_packet=spk,
                                    queue_num=qbase + 1)
                        if do_mk:
                            if l == 0:
                                nc.vector.tensor_tensor(
                                    out=mk[:],
                                    in0=dsts_sb[:, e0 // 128:e1 // 128]
                                        .to_broadcast([128, nch, 128]),
                                    in1=iota_d[:].rearrange("p (u c) -> p u c", u=1)
                                        .to_broadcast([128, nch, 128]),
                                    op=mybir.AluOpType.is_equal)
                                nc.sync.dma_start(
                                    out=masks_dram[:, e0 // 128:e1 // 128, :],
                                    in_=mk[:])
                            else:
                                nc.sync.dma_start(
                                    out=mk[:],
                                    in_=masks_dram[:, e0 // 128:e1 // 128, :])

                        # z = g + xr[dst] via bucket-broadcast; pad tail stays g
                        for b in range(2 if edge_stage >= 2 else 0):
                            for (k, d0, m) in runs:
                                off = int(layout["slot_edge_off"][d0] - e0)
                                base = b * et + off
                                if m == 1:
                                    nc.vector.tensor_scalar(
                                        out=gt[:, base:base + k],
                                        in0=gt[:, base:base + k],
                                        scalar1=xrTf[b][:, d0:d0 + 1],
                                        scalar2=None, op0=mybir.AluOpType.add)
                                else:
                                    seg = gt[:, base:base + m * k] \
                                        .rearrange("p (k m) -> p k m", m=m)
                                    nc.vector.tensor_tensor(
                                        out=seg, in0=seg,
                                        in1=xrT[b][:, d0:d0 + m]
                                            .to_broadcast([128, m, k])
                                            .rearrange("p m k -> p k m"),
                                        op=mybir.AluOpType.add)
                            nc.scalar.activation(
                                out=za[:, b * et:(b + 1) * et],
                                in_=gt[:, b * et:(b + 1) * et],
                                func=mybir.ActivationFunctionType.Abs)

                        # logits + exp, 512-edge chunks
                        for ch0 in range(0, et if edge_stage >= 3 else 0, 512):
                            cw = min(512, et - ch0)
                            lg = psp.tile([128, 512], fp32, tag="lg", name="lg", bufs=3)
                            nc.tensor.matmul(out=lg[:16, :cw], lhsT=a4z_s[:, 0, 0:16],
                                             rhs=gt[:, ch0:ch0 + cw],
                                             start=True, stop=False)
                            nc.tensor.matmul(out=lg[:16, :cw], lhsT=a4z_s[:, 1, 0:16],
                                             rhs=gt[:, et + ch0:et + ch0 + cw],
                                             start=False, stop=False)
                            nc.tensor.matmul(out=lg[:16, :cw], lhsT=a4a_s[:, 0, 0:16],
                                             rhs=za[:, ch0:ch0 + cw],
                                             start=False, stop=False)
                            nc.tensor.matmul(out=lg[:16, :cw], lhsT=a4a_s[:, 1, 0:16],
                                             rhs=za[:, et + ch0:et + ch0 + cw],
                                             start=False, stop=True)
                            nc.scalar.activation(
                                out=w16[:16, ch0:ch0 + cw], in_=lg[:16, :cw],
                                func=mybir.ActivationFunctionType.Exp)

                        if edge_stage >= 3:
                            wt = psp.tile([128, (SUBMAX // 128) * 16], bf16,
                                          tag="wt", name="wt")
                            for n in range(nch):
                                nc.tensor.transpose(
                                    out=wt[:, n * 16:(n + 1) * 16],
                                    in_=w16[0:16, n * 128:(n + 1) * 128],
                                    identity=identj_sb[0:16, :])
                            pend_wn.append((wn, wt, nch))
                            if len(pend_wn) > 1:
                                wn_, wt_, nch_ = pend_wn.pop(0)
                                nc.vector.tensor_copy(wn_[:], wt_[:, :nch_ * 16])
                        pend.append(dict(t=t, nch=nch, gn=gn, wn=wn, mk=mk,
                                         pw=pw, ci0=chunk_i,
                                         nct=n_chunks_tile,
                                         last=(si == n_sub - 1)))
                        chunk_i += nch
                        if len(pend) > PIPE_LAG:
                            flush_b()
                while pend_wn:
                    wn_, wt_, nch_ = pend_wn.pop(0)
                    nc.vector.tensor_copy(wn_[:], wt_[:, :nch_ * 16])
                while pend:
                    flush_b()

                # ---- GraphNorm stats (global) + h + skip
                s12 = sp.tile([128, 4], fp32, tag="s12", name="s12")  # [S1b0 S2b0 S1b1 S2b1]
                sq = sp.tile([128, NL], fp32, tag="sq", name="sq")
                for b in range(2):
                    nc.vector.tensor_reduce(
                        out=s12[:, 2 * b:2 * b + 1], in_=outT[b][:],
                        axis=mybir.AxisListType.X, op=mybir.AluOpType.add)
                    nc.vector.tensor_tensor(out=sq[:], in0=outT[b][:],
                                            in1=outT[b][:],
                                            op=mybir.AluOpType.mult)
                    nc.vector.tensor_reduce(
                        out=s12[:, 2 * b + 1:2 * b + 2], in_=sq[:],
                        axis=mybir.AxisListType.X, op=mybir.AluOpType.add)
                nc.sync.dma_start(
                    out=st_in[:].rearrange("(b p) s -> p b s", b=2),
                    in_=s12[:].rearrange("p (b s) -> p b s", b=2))
                if do_coll:
                    nc.gpsimd.collective_compute(
                        "AllReduce", mybir.AluOpType.add,
                        replica_groups=groups, ins=[st_in[:]], outs=[st_out[:]])
                else:
                    nc.sync.dma_start(out=st_out[:], in_=st_in[:])
                s12g = sp.tile([128, 4], fp32, tag="s12g", name="s12g")
                nc.sync.dma_start(
                    out=s12g[:].rearrange("p (b s) -> p b s", b=2),
                    in_=st_out[:].rearrange("(b p) s -> p b s", b=2))

                c1 = sp.tile([128, 2], fp32, tag="c1", name="c1")
                var = sp.tile([128, 2], fp32, tag="var", name="var")
                rstd = sp.tile([128, 2], fp32, tag="rstd", name="rstd")
                f_ = sp.tile([128, 2], fp32, tag="f_", name="f_")
                for b in range(2):
                    S1 = s12g[:, 2 * b:2 * b + 1]
                    S2 = s12g[:, 2 * b + 1:2 * b + 2]
                    # c1 = A + B*S1
                    nc.vector.tensor_tensor(out=c1[:, b:b + 1],
                                            in0=S1, in1=cbB_s[:, b:b + 1],
                                            op=mybir.AluOpType.mult)
                    nc.vector.tensor_tensor(out=c1[:, b:b + 1],
                                            in0=c1[:, b:b + 1],
                                            in1=cbA_s[:, b:b + 1],
                                            op=mybir.AluOpType.add)
                    # var = S2/N + c1*(2*S1/N + c1)
                    nc.vector.tensor_scalar(
                        out=var[:, b:b + 1], in0=S1, scalar1=2.0 / N,
                        scalar2=None, op0=mybir.AluOpType.mult)
                    nc.vector.tensor_tensor(out=var[:, b:b + 1],
                                            in0=var[:, b:b + 1],
                                            in1=c1[:, b:b + 1],
                                            op=mybir.AluOpType.add)
                    nc.vector.tensor_tensor(out=var[:, b:b + 1],
                                            in0=var[:, b:b + 1],
                                            in1=c1[:, b:b + 1],
                                            op=mybir.AluOpType.mult)
                    nc.vector.tensor_scalar(
                        out=var[:, b:b + 1], in0=S2, scalar1=1.0 / N,
                        scalar2=var[:, b:b + 1], op0=mybir.AluOpType.mult,
                        op1=mybir.AluOpType.add)
                    # rstd = 1/sqrt(var + eps)
                    nc.scalar.activation(
                        out=rstd[:, b:b + 1], in_=var[:, b:b + 1],
                        func=mybir.ActivationFunctionType.Sqrt, bias=eps_t[:])
                    nc.vector.reciprocal(rstd[:, b:b + 1], rstd[:, b:b + 1])
                    nc.vector.tensor_tensor(out=f_[:, b:b + 1],
                                            in0=rstd[:, b:b + 1],
                                            in1=gnw_s[:, b:b + 1],
                                            op=mybir.AluOpType.mult)
                    # h = (out + c1) * f + gnb  (into hT)
                    nc.vector.tensor_scalar(
                        out=hT[b][:], in0=outT[b][:],
                        scalar1=c1[:, b:b + 1], scalar2=None,
                        op0=mybir.AluOpType.add)
                    nc.vector.tensor_scalar(
                        out=hT[b][:], in0=hT[b][:],
                        scalar1=f_[:, b:b + 1], scalar2=gnb_s[:, b:b + 1],
                        op0=mybir.AluOpType.mult, op1=mybir.AluOpType.add)
                    # elu(h) - 1 fold: h' = relu(h) + exp(min(h,0))
                    nc.vector.tensor_scalar(
                        out=sq[:], in0=hT[b][:], scalar1=0.0, scalar2=None,
                        op0=mybir.AluOpType.min)
                    nc.scalar.activation(
                        out=sq[:], in_=sq[:],
                        func=mybir.ActivationFunctionType.Exp)
                    nc.vector.tensor_scalar(
                        out=hT[b][:], in0=hT[b][:], scalar1=0.0, scalar2=None,
                        op0=mybir.AluOpType.max)
                    nc.vector.tensor_tensor(
                        out=hT[b][:], in0=hT[b][:], in1=sq[:],
                        op=mybir.AluOpType.add)

                # skip matmul: xT += W_sk.T h' + bsk_eff
                for bo in range(2):
                    for ch0 in range(0, NL, 512):
                        cw = min(512, NL - ch0)
                        ps = psp.tile([128, 512], fp32, tag="wsum", name="wsum")
                        for bi in range(2):
                            nc.tensor.matmul(
                                out=ps[:, :cw],
                                lhsT=wsk_s[:, bi, bo],
                                rhs=hT[bi][:, ch0:ch0 + cw],
                                start=(bi == 0), stop=(bi == 1))
                        nc.vector.tensor_tensor(
                            out=xT[bo][:, ch0:ch0 + cw],
                            in0=xT[bo][:, ch0:ch0 + cw], in1=ps[:, :cw],
                            op=mybir.AluOpType.add)
                    nc.vector.tensor_scalar(
                        out=xT[bo][:], in0=xT[bo][:],
                        scalar1=bsk_s[:, bo:bo + 1], scalar2=None,
                        op0=mybir.AluOpType.add)
                    nc.vector.tensor_copy(xTb[bo][:], xT[bo][:])

            fcw_s = pp.tile([128, 2], bf16, tag="fcws", name="fcws")
            fcb_s = pp.tile([1, 1], fp32, tag="fcbs", name="fcbs")
            nc.sync.dma_start(out=fcw_s[:], in_=v_pb2("fcw_t"))
            nc.sync.dma_start(out=fcb_s[:], in_=fcb_ap)

            for l in range(n_layers):
                layer(l)

            # final fc
            y_sb = sp.tile([1, NL], fp32, tag="ysb", name="ysb")
            for ch0 in range(0, NL, 512):
                cw = min(512, NL - ch0)
                ps = psp.tile([128, 512], fp32, tag="lg", name="lg", bufs=3)
                for b in range(2):
                    nc.tensor.matmul(out=ps[:1, :cw], lhsT=fcw_s[:, b:b + 1],
                                     rhs=xTb[b][:, ch0:ch0 + cw],
                                     start=(b == 0), stop=(b == 1))
                nc.scalar.activation(
                    out=y_sb[:, ch0:ch0 + cw], in_=ps[:1, :cw],
                    func=mybir.ActivationFunctionType.Identity,
                    bias=fcb_s[:], scale=1.0)
            nc.sync.dma_start(out=y_out[:], in_=y_sb[:])

    nc.compile()
    return nc


# ---------------------------------------------------------------- runner

_CACHE = {}


def kernel(**inputs):
    layout, in_maps = prep_inputs(inputs)
    key = (layout["NL"], layout["E_pad"],
           tuple(int(x) for x in layout["tile_edge_base"]))
    if key not in _CACHE:
        _CACHE[key] = build_program(layout)
    nc = _CACHE[key]
    res = bass_utils.run_bass_kernel_spmd(nc, in_maps, core_ids=list(range(NCORES)))
    y = np.zeros(N, np.float32)
    for c in range(NCORES):
        yc = np.asarray(res.results[c]["y"], np.float32).reshape(-1)
        y[layout["core_nodes"][c]] = yc[layout["cores"][c]["slot_of_node"]]
    return y


if __name__ == "__main__":
    sys.path.insert(0, "/root/problem")
    import jax
    import reference

    with jax.default_device(jax.devices("cpu")[0]):
        inputs = {k: np.asarray(v) for k, v in reference.setup_inputs().items()}
        expected = np.asarray(reference.reference(**inputs))
    got = kernel(**inputs)
    rel = np.linalg.norm(got - expected) / np.linalg.norm(expected)
    print("rel l2:", rel)
    print(expected[:4], got[:4])

